# revision 19
# baseline (speedup 1.0000x reference)
"""CrossAttention2D TRN2 Bass kernel — data-parallel over batch on 8 NeuronCores.

Per core (one batch element), computed fully transposed ("feature-major"):
  qT[j,n]  = Wq'[c,j].T @ x[c,n]            (Wq' = Wq*scale)
  kT[j,t]  = Wk[c,j].T @ ctxT[c,t]
  v[t,j]   = ctxT[c,t].T @ Wv[c,j]
  ST[t,n]  = kT_h[d,t].T @ qT_h[d,n]        (per head, K=64)
  PT[t,n]  = exp(ST + maskbias[t])          (mask folds into ACT per-partition bias)
  OT       = [1(x64) | v_h][t,128].T @ PT[t,n]
             rows 0:64 = softmax denominator replicated, rows 64:128 = raw O
  oT[d,n]  = OT[64:128] * recip_fast(OT[0:64])   (one DVE op each)
  outT[co,n] = Wo[j,co].T @ oT[j,n]

v2 schedule (vs baseline):
  - Head pairs: the two K=64 ST matmuls of heads (2p, 2p+1) are issued
    back-to-back at PE row offsets 0/64 so they genuinely co-execute.
  - Fused exp: one ACTIVATE over [128, 2048] psum (both heads of a pair,
    one tcc) instead of two [128,1024] — cuts ACT fixed cost.
  - qT chunks jc3-5, v half 1, and v half 0 are interleaved INTO the
    attention pair loop as "fill units" so PE chews projection work while
    ACT exps run (attention phase is otherwise ACT-bound).
  - DMA: ctx+wk(jc01) first so kT starts ~10us; x/wq-lo interleaved per-kc
    so qT streams at arrival pace; wv/wq-hi/wo later (JIT deadlines).
  - Copies routed off ACT during attention (DVE), on ACT elsewhere.
  - PSUM: "st" pool 1x[128,2048] (4 banks) + "mm" pool 2x[128,1024] (4 banks).
Output fp16, upcast on host.
"""

import sys

for _p in ("/opt/trn_rl_repo", "/opt/pypackages"):
    if _p not in sys.path:
        sys.path.append(_p)

import numpy as np

import concourse.bass as bass
import concourse.tile as tile
from concourse import bacc, mybir
from concourse.bass_utils import run_bass_kernel_spmd

F32 = mybir.dt.float32
F16 = mybir.dt.float16

B = 8
C = 768            # DIM (q/out features)
CTX = 1024         # CTX_DIM
T = 256            # context tokens
N = 1024           # image tokens (32*32)
NH = 12            # heads
D = 64             # head dim
P = 128
CC = C // P        # 6
TC = T // P        # 2
KC = CTX // P      # 8
MASK_NEG = -30.0

_NC_CACHE = {}


def _build_nc():
    if "nc" in _NC_CACHE:
        return _NC_CACHE["nc"]

    MDT = F16

    nc = bacc.Bacc("TRN2", target_bir_lowering=False, debug=False)

    x_d = nc.dram_tensor("x", [C, N], MDT, kind="ExternalInput")
    ctxT_d = nc.dram_tensor("ctxT", [CTX, T], MDT, kind="ExternalInput")
    mb_d = nc.dram_tensor("mb", [P, TC], F32, kind="ExternalInput")
    wq_d = nc.dram_tensor("wq", [C, C], MDT, kind="ExternalInput")
    # wk host-permuted to [jc, p, kc*128] so each per-jc DMA piece reads
    # 2KB-contiguous DRAM lines per partition
    wk_d = nc.dram_tensor("wk", [CC, P, CTX], MDT, kind="ExternalInput")
    wv_d = nc.dram_tensor("wv", [CTX, C], MDT, kind="ExternalInput")
    wo_d = nc.dram_tensor("wo", [C, C], MDT, kind="ExternalInput")
    out_d = nc.dram_tensor("out", [C, N], F16, kind="ExternalOutput")

    wq_r = wq_d.rearrange("(k p) m -> p k m", p=P)
    x_r = x_d.rearrange("(k p) n -> p k n", p=P)
    ctx_r = ctxT_d.rearrange("(k p) t -> p k t", p=P)
    wk_r = wk_d.rearrange("j p (k m) -> j p k m", m=P)
    wv_r = wv_d.rearrange("(k p) m -> p k m", p=P)
    wo_r = wo_d.rearrange("(k p) m -> p k m", p=P)
    out_r = out_d.rearrange("(k p) n -> p k n", p=P)

    with tile.TileContext(nc) as tc:
        with (
            tc.tile_pool(name="consts", bufs=1) as cpool,
            tc.tile_pool(name="acts", bufs=1) as apool,
            tc.tile_pool(name="s1", bufs=1) as s1pool,
            tc.tile_pool(name="pt", bufs=2) as ptpool,
            tc.tile_pool(name="rec", bufs=4) as rpool,
            tc.tile_pool(name="outsb", bufs=3) as opool,
            tc.tile_pool(name="pst", bufs=2, space="PSUM") as pst,
            tc.tile_pool(name="pmm", bufs=2, space="PSUM") as pmm,
        ):
            # ---- SBUF input tiles ----
            x_sb = s1pool.tile([P, CC, N], MDT)
            wq_sb = s1pool.tile([P, CC, C], MDT)
            ctxT_sb = s1pool.tile([P, KC, T], MDT)
            wk_sb = s1pool.tile([P, CC, KC, P], MDT)   # (p, jc, kc, m)
            wv_sb = s1pool.tile([P, KC, C], MDT)
            wo_sb = s1pool.tile([P, CC, C], MDT)
            mb_sb = cpool.tile([P, TC], F32)

            # ---- DMA issue order (per-queue order = arrival order).
            # sync:   ctx04 wkj0 [x0 wq1lo x2 wq3lo x4 wq5lo] wkj2 wkj4
            #         wv0a wv1a wqhi_a wo_a
            # scalar: ctx48 wkj1 [x1 wq0lo x3 wq2lo x5 wq4lo] wkj3 wkj5
            #         wv0b wv1b wqhi_b wo_b
            H1 = 384   # wq/wv column split: cols 0:384 = jc0-2 / heads 0-5
            wk_pr = wk_d.rearrange("j p (k m) -> p j k m", m=P)
            with tc.high_priority():
                # consolidated pieces; wk j0/j1 first so kT starts earliest
                nc.sync.dma_start(wk_sb[:, 0:1, :, :], wk_pr[:, 0:1])
                nc.scalar.dma_start(wk_sb[:, 1:2, :, :], wk_pr[:, 1:2])
                nc.sync.dma_start(ctxT_sb[:, 0:4, :], ctx_r[:, 0:4, :])
                nc.scalar.dma_start(ctxT_sb[:, 4:8, :], ctx_r[:, 4:8, :])
                nc.sync.dma_start(wk_sb[:, 2:4, :, :], wk_pr[:, 2:4])
                nc.scalar.dma_start(wk_sb[:, 4:6, :, :], wk_pr[:, 4:6])
                nc.scalar.dma_start(wq_sb[:, :, 0:H1], wq_r[:, :, 0:H1])
                nc.sync.dma_start(x_sb[:, 0:2, :], x_r[:, 0:2, :])
                nc.scalar.dma_start(x_sb[:, 2:4, :], x_r[:, 2:4, :])
                nc.sync.dma_start(x_sb[:, 4:6, :], x_r[:, 4:6, :])
                # wv half 0 (heads 0-5), split across queues by kc
                nc.sync.dma_start(wv_sb[:, 0:4, 0:H1], wv_r[:, 0:4, 0:H1])
                nc.scalar.dma_start(wv_sb[:, 4:8, 0:H1], wv_r[:, 4:8, 0:H1])
                # wv half 1 (heads 6-11)
                nc.sync.dma_start(wv_sb[:, 0:4, H1:C], wv_r[:, 0:4, H1:C])
                nc.scalar.dma_start(wv_sb[:, 4:8, H1:C], wv_r[:, 4:8, H1:C])
                # wq-hi (jc3-5)
                nc.sync.dma_start(wq_sb[:, :, H1:C], wq_r[:, :, H1:C])
                # wo
                nc.scalar.dma_start(wo_sb[:, :, 0:H1], wo_r[:, :, 0:H1])
                nc.sync.dma_start(wo_sb[:, :, H1:C], wo_r[:, :, H1:C])
                nc.gpsimd.dma_start(mb_sb[:], mb_d[:])

            # ---- activation tiles ----
            qT = apool.tile([P, CC, N], MDT)
            kT = apool.tile([P, CC, T], MDT)
            vT = apool.tile([P, TC, NH, D], MDT)
            oT = apool.tile([P, CC, N], MDT)
            ones_sb = cpool.tile([P, D], MDT)
            nc.gpsimd.memset(ones_sb[:], 1.0)

            # ---- PE warm-up junk ----
            warm = cpool.tile([P, 256], MDT)
            nc.vector.memset(warm[:], 0.0)
            for w in range(6):
                wps = pmm.tile([P, N], F32, tag="mm", name=f"warm{w // 3}")
                nc.tensor.matmul(wps[:, 0:256], warm[:, 0:P], warm[:],
                                 start=True, stop=True)

            # ---- stage helpers (self-contained psum units) ----
            def kt_chunk(jc):
                ps = pmm.tile([P, N], F32, tag="mm", name=f"k{jc}")
                for kc in range(KC):
                    nc.tensor.matmul(
                        ps[:, 0:T],
                        wk_sb[:, jc, kc, :],
                        ctxT_sb[:, kc, :],
                        start=(kc == 0),
                        stop=(kc == KC - 1),
                    )
                nc.vector.tensor_copy(kT[:, jc, :], ps[:, 0:T])

            def q_half(jc, nh):
                # qT[:, jc, nh*512:(nh+1)*512], 6 kc-accumulating mms
                ps = pmm.tile([P, N], F32, tag="mm", name=f"q{jc}_{nh}")
                for kc in range(CC):
                    nc.tensor.matmul(
                        ps[:, 0:512],
                        wq_sb[:, kc, jc * P:(jc + 1) * P],
                        x_sb[:, kc, nh * 512:(nh + 1) * 512],
                        start=(kc == 0),
                        stop=(kc == CC - 1),
                    )
                nc.vector.tensor_copy(qT[:, jc, nh * 512:(nh + 1) * 512],
                                      ps[:, 0:512])

            def v_unit(jh, tcc):
                # heads 6jh..6jh+5, one t-chunk
                ps = pmm.tile([P, N], F32, tag="mm", name=f"v{jh}_{tcc}")
                for kc in range(KC):
                    nc.tensor.matmul(
                        ps[:, 0:H1],
                        ctxT_sb[:, kc, tcc * P:(tcc + 1) * P],
                        wv_sb[:, kc, jh * H1:(jh + 1) * H1],
                        start=(kc == 0),
                        stop=(kc == KC - 1),
                    )
                nc.vector.tensor_copy(
                    vT[:, tcc, 6 * jh:6 * jh + 6, :],
                    ps[:, 0:H1].rearrange("p (h d) -> p h d", d=D),
                )

            # fill units consumed inside the attention pair loop
            fills = [
                lambda: q_half(3, 0),
                lambda: q_half(3, 1),
                lambda: v_unit(1, 0),
                lambda: v_unit(1, 1),
                lambda: q_half(4, 0),
                lambda: q_half(4, 1),
                lambda: q_half(5, 0),
                lambda: q_half(5, 1),
            ]
            FILL_PLAN = [2, 2, 2, 1, 1, 0]   # units consumed per pair
            fi = [0]

            def fill(k=1):
                for _ in range(k):
                    if fi[0] < len(fills):
                        fills[fi[0]]()
                        fi[0] += 1

            def st_stage(p, tcc, pts):
                # per-head score tiles; the two heads' K=64 mms issue
                # adjacently at PE row offsets 0/64 so they co-execute
                sts = [pst.tile([P, N], F32, tag="st", name=f"st{hl}")
                       for hl in range(2)]
                for nh in range(2):
                    for hl in range(2):
                        r0 = hl * D
                        nc.tensor.matmul(
                            sts[hl][:, nh * 512:(nh + 1) * 512],
                            kT[r0:r0 + D, p, tcc * P:(tcc + 1) * P],
                            qT[r0:r0 + D, p, nh * 512:(nh + 1) * 512],
                            start=True,
                            stop=True,
                        )
                for hl in range(2):
                    nc.scalar.activation(
                        pts[hl][:, tcc, :],
                        sts[hl][:],
                        mybir.ActivationFunctionType.Exp,
                        bias=mb_sb[:, tcc:tcc + 1],
                    )

            def denotp_stage(p, pts):
                # denominators: den[hl*64:...] = ones.T @ PT_hl — col-tiled
                # pair (positions (0,0)/(0,64)) sharing banks row-disjoint
                den = pmm.tile([P, N], F32, tag="mm", name="den")
                for tcc in range(TC):
                    for nh in range(2):
                        for hl in range(2):
                            nc.tensor.matmul(
                                den[hl * D:(hl + 1) * D, nh * 512:(nh + 1) * 512],
                                ones_sb[:],
                                pts[hl][:, tcc, nh * 512:(nh + 1) * 512],
                                start=(tcc == 0),
                                stop=(tcc == TC - 1),
                            )
                # raw attention outputs, col-tiled the same way
                otp = pmm.tile([P, N], F32, tag="mm", name="otp")
                for tcc in range(TC):
                    for nh in range(2):
                        for hl in range(2):
                            nc.tensor.matmul(
                                otp[hl * D:(hl + 1) * D, nh * 512:(nh + 1) * 512],
                                vT[:, tcc, 2 * p + hl, :],
                                pts[hl][:, tcc, nh * 512:(nh + 1) * 512],
                                start=(tcc == 0),
                                stop=(tcc == TC - 1),
                            )
                # one full-lane reciprocal + one full-lane multiply per pair
                rec = rpool.tile([P, N], F32, tag="rec")
                nc.vector.reciprocal_approx_fast(rec[:], den[:])
                nc.vector.tensor_mul(oT[:, p, :], otp[:], rec[:])

            # ---- pre-attention: kT, qT jc0-2, v half 0 ----
            kt_chunk(0)
            kt_chunk(1)
            q_half(0, 0)
            q_half(0, 1)
            q_half(1, 0)
            q_half(1, 1)
            kt_chunk(2)
            kt_chunk(3)
            q_half(2, 0)
            q_half(2, 1)
            kt_chunk(4)
            kt_chunk(5)
            v_unit(0, 0)
            v_unit(0, 1)

            # ACT exp-table preload: walrus inserts the table load before the
            # first ACTIVATE; keep that off the DMA-issue window
            dume = cpool.tile([P, 8], MDT)
            nc.scalar.activation(dume[:], warm[:, 0:8],
                                 mybir.ActivationFunctionType.Exp)

            # ---- attention: 6 head-pairs, software-pipelined so pair p-1's
            # den/otp matmuls run under pair p's exp windows ----
            pending = None
            for p in range(CC):
                pts = [ptpool.tile([P, TC, N], MDT, tag="pt",
                                   name=f"pt{(2 * p + hl) % 4}")
                       for hl in range(2)]
                st_stage(p, 0, pts)
                if pending is not None:
                    denotp_stage(*pending)
                st_stage(p, 1, pts)
                fill(FILL_PLAN[p])
                pending = (p, pts)
            denotp_stage(*pending)

            # ---- out projection: outT = Wo.T @ oT ----
            # last oc as two separate 1-bank half-tiles so the final
            # copy+DMA tail overlaps the last matmuls
            oqs = [nc.sync, nc.scalar]
            for oc in range(CC - 1):
                ps = pmm.tile([P, N], F32, tag="mm", name=f"o{oc % 2}")
                for jc in range(CC):
                    for nh in range(2):
                        nc.tensor.matmul(
                            ps[:, nh * 512:(nh + 1) * 512],
                            wo_sb[:, jc, oc * P:(oc + 1) * P],
                            oT[:, jc, nh * 512:(nh + 1) * 512],
                            start=(jc == 0),
                            stop=(jc == CC - 1),
                        )
                ob = opool.tile([P, N], F16, tag="ob")
                nc.scalar.copy(ob[:, 0:512], ps[:, 0:512])
                nc.vector.tensor_copy(ob[:, 512:1024], ps[:, 512:1024])
                oqs[oc % 2].dma_start(out_r[:, oc, 0:512], ob[:, 0:512])
                oqs[(oc + 1) % 2].dma_start(
                    out_r[:, oc, 512:1024], ob[:, 512:1024])
            oc = CC - 1
            ob = opool.tile([P, N], F16, tag="ob")
            for nh in range(2):
                ph = pmm.tile([P, 512], F32, tag="mm", name=f"oh{nh}")
                for jc in range(CC):
                    nc.tensor.matmul(
                        ph[:],
                        wo_sb[:, jc, oc * P:(oc + 1) * P],
                        oT[:, jc, nh * 512:(nh + 1) * 512],
                        start=(jc == 0),
                        stop=(jc == CC - 1),
                    )
                sl = slice(nh * 512, (nh + 1) * 512)
                if nh == 0:
                    nc.scalar.copy(ob[:, sl], ph[:])
                else:
                    nc.vector.tensor_copy(ob[:, sl], ph[:])
                oqs[nh].dma_start(out_r[:, oc, sl], ob[:, sl])

    nc.compile()
    _NC_CACHE["nc"] = nc
    return nc


def kernel(x, context, context_mask, Wq, bq, Wk, bk, Wv, bv, Wo, bo,
           _trace=False):
    np_dt = np.float16
    x = np.asarray(x, dtype=np.float32)
    context = np.asarray(context, dtype=np.float32)
    context_mask = np.asarray(context_mask)
    scale = float(D) ** -0.5
    wq = np.ascontiguousarray(np.asarray(Wq) * scale).astype(np_dt)
    # wk permuted to [jc, p, kc*128]: wk_p[j, p, k*128+m] = Wk[k*128+p, j*128+m]
    wk = np.ascontiguousarray(
        np.asarray(Wk).reshape(KC, P, CC, P).transpose(2, 1, 0, 3)
        .reshape(CC, P, CTX)).astype(np_dt)
    wv = np.ascontiguousarray(np.asarray(Wv)).astype(np_dt)
    wo = np.ascontiguousarray(np.asarray(Wo)).astype(np_dt)

    in_maps = []
    for b in range(B):
        mb = np.where(context_mask[b] != 0, 0.0, MASK_NEG).astype(np.float32)
        im = {
            "x": np.ascontiguousarray(x[b].reshape(C, N)).astype(np_dt),
            "ctxT": np.ascontiguousarray(context[b].T).astype(np_dt),
            "mb": np.ascontiguousarray(mb.reshape(TC, P).T),
            "wq": wq, "wk": wk, "wv": wv, "wo": wo,
        }
        in_maps.append(im)

    nc = _build_nc()
    try:
        res = run_bass_kernel_spmd(nc, in_maps, list(range(B)), trace=_trace)
    except Exception:
        # transient NRT_EXEC_UNIT_UNRECOVERABLE etc. — one retry
        res = run_bass_kernel_spmd(nc, in_maps, list(range(B)), trace=_trace)
    out = np.stack([
        res.results[b]["out"].astype(np.float32).reshape(C, 32, 32)
        for b in range(B)
    ])
    if _trace:
        kernel.last_exec_time_ns = res.exec_time_ns
        kernel.last_results = res
    return out


# revision 21
# speedup vs baseline: 1.0807x; 1.0807x over previous
"""CrossAttention2D TRN2 Bass kernel — data-parallel over batch on 8 NeuronCores.

Per core (one batch element), computed fully transposed ("feature-major"):
  qT[j,n]  = Wq'[c,j].T @ x[c,n]            (Wq' = Wq*scale)
  kT[j,t]  = Wk[c,j].T @ ctxT[c,t]
  v[t,j]   = ctxT[c,t].T @ Wv[c,j]
  ST[t,n]  = kT_h[d,t].T @ qT_h[d,n]        (per head, K=64)
  PT[t,n]  = exp(ST + maskbias[t])          (mask folds into ACT per-partition bias)
  OT       = [1(x64) | v_h][t,128].T @ PT[t,n]
             rows 0:64 = softmax denominator replicated, rows 64:128 = raw O
  oT[d,n]  = OT[64:128] * recip_fast(OT[0:64])   (one DVE op each)
  outT[co,n] = Wo[j,co].T @ oT[j,n]

v2 schedule (vs baseline):
  - Head pairs: the two K=64 ST matmuls of heads (2p, 2p+1) are issued
    back-to-back at PE row offsets 0/64 so they genuinely co-execute.
  - Fused exp: one ACTIVATE over [128, 2048] psum (both heads of a pair,
    one tcc) instead of two [128,1024] — cuts ACT fixed cost.
  - qT chunks jc3-5, v half 1, and v half 0 are interleaved INTO the
    attention pair loop as "fill units" so PE chews projection work while
    ACT exps run (attention phase is otherwise ACT-bound).
  - DMA: ctx+wk(jc01) first so kT starts ~10us; x/wq-lo interleaved per-kc
    so qT streams at arrival pace; wv/wq-hi/wo later (JIT deadlines).
  - Copies routed off ACT during attention (DVE), on ACT elsewhere.
  - PSUM: "st" pool 1x[128,2048] (4 banks) + "mm" pool 2x[128,1024] (4 banks).
Output fp16, upcast on host.
"""

import sys

for _p in ("/opt/trn_rl_repo", "/opt/pypackages"):
    if _p not in sys.path:
        sys.path.append(_p)

import numpy as np

import concourse.bass as bass
import concourse.tile as tile
from concourse import bacc, mybir
from concourse.bass_utils import run_bass_kernel_spmd

F32 = mybir.dt.float32
F16 = mybir.dt.float16

B = 8
C = 768            # DIM (q/out features)
CTX = 1024         # CTX_DIM
T = 256            # context tokens
N = 1024           # image tokens (32*32)
NH = 12            # heads
D = 64             # head dim
P = 128
CC = C // P        # 6
TC = T // P        # 2
KC = CTX // P      # 8
MASK_NEG = -30.0

_NC_CACHE = {}


def _build_nc():
    if "nc" in _NC_CACHE:
        return _NC_CACHE["nc"]

    MDT = F16

    nc = bacc.Bacc("TRN2", target_bir_lowering=False, debug=False)

    x_d = nc.dram_tensor("x", [C, N], MDT, kind="ExternalInput")
    ctxT_d = nc.dram_tensor("ctxT", [CTX, T], MDT, kind="ExternalInput")
    mb_d = nc.dram_tensor("mb", [P, TC], F32, kind="ExternalInput")
    wq_d = nc.dram_tensor("wq", [C, C], MDT, kind="ExternalInput")
    # wk host-permuted to [jc, p, kc*128] so each per-jc DMA piece reads
    # 2KB-contiguous DRAM lines per partition
    wk_d = nc.dram_tensor("wk", [CC, P, CTX], MDT, kind="ExternalInput")
    wv_d = nc.dram_tensor("wv", [CTX, C], MDT, kind="ExternalInput")
    wo_d = nc.dram_tensor("wo", [C, C], MDT, kind="ExternalInput")
    out_d = nc.dram_tensor("out", [C, N], F16, kind="ExternalOutput")

    wq_r = wq_d.rearrange("(k p) m -> p k m", p=P)
    x_r = x_d.rearrange("(k p) n -> p k n", p=P)
    ctx_r = ctxT_d.rearrange("(k p) t -> p k t", p=P)
    wk_r = wk_d.rearrange("j p (k m) -> j p k m", m=P)
    wv_r = wv_d.rearrange("(k p) m -> p k m", p=P)
    wo_r = wo_d.rearrange("(k p) m -> p k m", p=P)
    out_r = out_d.rearrange("(k p) n -> p k n", p=P)

    with tile.TileContext(nc) as tc:
        with (
            tc.tile_pool(name="consts", bufs=1) as cpool,
            tc.tile_pool(name="acts", bufs=1) as apool,
            tc.tile_pool(name="s1", bufs=1) as s1pool,
            tc.tile_pool(name="pt", bufs=4) as ptpool,
            tc.tile_pool(name="rec", bufs=4) as rpool,
            tc.tile_pool(name="outsb", bufs=3) as opool,
            tc.tile_pool(name="pst", bufs=2, space="PSUM") as pst,
            tc.tile_pool(name="pmm", bufs=2, space="PSUM") as pmm,
        ):
            # ---- SBUF input tiles ----
            x_sb = s1pool.tile([P, CC, N], MDT)
            wq_sb = s1pool.tile([P, CC, C], MDT)
            ctxT_sb = s1pool.tile([P, KC, T], MDT)
            wk_sb = s1pool.tile([P, CC, KC, P], MDT)   # (p, jc, kc, m)
            wv_sb = s1pool.tile([P, KC, C], MDT)
            wo_sb = s1pool.tile([P, CC, C], MDT)
            mb_sb = cpool.tile([P, TC], F32)

            # ---- DMA issue order (per-queue order = arrival order).
            # sync:   ctx04 wkj0 [x0 wq1lo x2 wq3lo x4 wq5lo] wkj2 wkj4
            #         wv0a wv1a wqhi_a wo_a
            # scalar: ctx48 wkj1 [x1 wq0lo x3 wq2lo x5 wq4lo] wkj3 wkj5
            #         wv0b wv1b wqhi_b wo_b
            H1 = 384   # wq/wv column split: cols 0:384 = jc0-2 / heads 0-5
            wk_pr = wk_d.rearrange("j p (k m) -> p j k m", m=P)
            with tc.high_priority():
                # consolidated pieces; wk j0/j1 first so kT starts earliest
                nc.sync.dma_start(wk_sb[:, 0:1, :, :], wk_pr[:, 0:1])
                nc.scalar.dma_start(wk_sb[:, 1:2, :, :], wk_pr[:, 1:2])
                nc.sync.dma_start(ctxT_sb[:, 0:4, :], ctx_r[:, 0:4, :])
                nc.scalar.dma_start(ctxT_sb[:, 4:8, :], ctx_r[:, 4:8, :])
                nc.sync.dma_start(wk_sb[:, 2:4, :, :], wk_pr[:, 2:4])
                nc.scalar.dma_start(wk_sb[:, 4:6, :, :], wk_pr[:, 4:6])
                nc.scalar.dma_start(wq_sb[:, :, 0:H1], wq_r[:, :, 0:H1])
                nc.sync.dma_start(x_sb[:, 0:2, :], x_r[:, 0:2, :])
                nc.scalar.dma_start(x_sb[:, 2:4, :], x_r[:, 2:4, :])
                nc.sync.dma_start(x_sb[:, 4:6, :], x_r[:, 4:6, :])
                # wv half 0 (heads 0-5), split across queues by kc
                nc.sync.dma_start(wv_sb[:, 0:4, 0:H1], wv_r[:, 0:4, 0:H1])
                nc.scalar.dma_start(wv_sb[:, 4:8, 0:H1], wv_r[:, 4:8, 0:H1])
                # wv half 1 (heads 6-11)
                nc.sync.dma_start(wv_sb[:, 0:4, H1:C], wv_r[:, 0:4, H1:C])
                nc.scalar.dma_start(wv_sb[:, 4:8, H1:C], wv_r[:, 4:8, H1:C])
                # wq-hi (jc3-5)
                nc.sync.dma_start(wq_sb[:, :, H1:C], wq_r[:, :, H1:C])
                # wo
                nc.scalar.dma_start(wo_sb[:, :, 0:H1], wo_r[:, :, 0:H1])
                nc.sync.dma_start(wo_sb[:, :, H1:C], wo_r[:, :, H1:C])
                nc.gpsimd.dma_start(mb_sb[:], mb_d[:])

            # ---- activation tiles ----
            qT = apool.tile([P, CC, N], MDT)
            kT = apool.tile([P, CC, T], MDT)
            vT = apool.tile([P, TC, NH, D], MDT)
            oT = apool.tile([P, CC, N], MDT)
            ones_sb = cpool.tile([P, D], MDT)
            nc.gpsimd.memset(ones_sb[:], 1.0)

            # ---- PE warm-up junk ----
            warm = cpool.tile([P, 256], MDT)
            nc.vector.memset(warm[:], 0.0)
            for w in range(22):
                wps = pmm.tile([P, N], F32, tag="mm", name=f"warm{w % 4}")
                nc.tensor.matmul(wps[:, 0:256], warm[:, 0:P], warm[:],
                                 start=True, stop=True)

            # ---- stage helpers (self-contained psum units) ----
            def ptile(pool, name):
                return pool.tile([P, N], F32,
                                 tag="mm" if pool is pmm else "st", name=name)

            def kt_chunk(jc, pool=None):
                ps = ptile(pool or pmm, f"k{jc}")
                for kc in range(KC):
                    nc.tensor.matmul(
                        ps[:, 0:T],
                        wk_sb[:, jc, kc, :],
                        ctxT_sb[:, kc, :],
                        start=(kc == 0),
                        stop=(kc == KC - 1),
                    )
                nc.vector.tensor_copy(kT[:, jc, :], ps[:, 0:T])

            def q_half(jc, nh, pool=None):
                # qT[:, jc, nh*512:(nh+1)*512], 6 kc-accumulating mms
                ps = ptile(pool or pmm, f"q{jc}_{nh}")
                for kc in range(CC):
                    nc.tensor.matmul(
                        ps[:, 0:512],
                        wq_sb[:, kc, jc * P:(jc + 1) * P],
                        x_sb[:, kc, nh * 512:(nh + 1) * 512],
                        start=(kc == 0),
                        stop=(kc == CC - 1),
                    )
                nc.vector.tensor_copy(qT[:, jc, nh * 512:(nh + 1) * 512],
                                      ps[:, 0:512])

            def v_unit(jh, tcc, pool=None):
                # heads 6jh..6jh+5, one t-chunk
                ps = ptile(pool or pmm, f"v{jh}_{tcc}")
                for kc in range(KC):
                    nc.tensor.matmul(
                        ps[:, 0:H1],
                        ctxT_sb[:, kc, tcc * P:(tcc + 1) * P],
                        wv_sb[:, kc, jh * H1:(jh + 1) * H1],
                        start=(kc == 0),
                        stop=(kc == KC - 1),
                    )
                nc.vector.tensor_copy(
                    vT[:, tcc, 6 * jh:6 * jh + 6, :],
                    ps[:, 0:H1].rearrange("p (h d) -> p h d", d=D),
                )

            # fill units consumed inside the attention pair loop
            fills = [
                lambda: v_unit(0, 0),
                lambda: v_unit(0, 1),
                lambda: q_half(3, 0),
                lambda: q_half(3, 1),
                lambda: v_unit(1, 0),
                lambda: v_unit(1, 1),
                lambda: q_half(4, 0),
                lambda: q_half(4, 1),
                lambda: q_half(5, 0),
                lambda: q_half(5, 1),
            ]
            FILL_PLAN = [2, 2, 2, 2, 2, 0]   # units consumed per pair
            fi = [0]

            def fill(k=1):
                for _ in range(k):
                    if fi[0] < len(fills):
                        fills[fi[0]]()
                        fi[0] += 1

            def st_stage(p, tcc, pts):
                # per-head score tiles; the two heads' K=64 mms issue
                # adjacently at PE row offsets 0/64 so they co-execute
                sts = [pst.tile([P, N], F32, tag="st", name=f"st{hl}")
                       for hl in range(2)]
                for nh in range(2):
                    for hl in range(2):
                        r0 = hl * D
                        nc.tensor.matmul(
                            sts[hl][:, nh * 512:(nh + 1) * 512],
                            kT[r0:r0 + D, p, tcc * P:(tcc + 1) * P],
                            qT[r0:r0 + D, p, nh * 512:(nh + 1) * 512],
                            start=True,
                            stop=True,
                        )
                for hl in range(2):
                    nc.scalar.activation(
                        pts[hl][:, tcc, :],
                        sts[hl][:],
                        mybir.ActivationFunctionType.Exp,
                        bias=mb_sb[:, tcc:tcc + 1],
                    )

            def denotp_stage(p, pts):
                # denominators: den[hl*64:...] = ones.T @ PT_hl — col-tiled
                # pair (positions (0,0)/(0,64)) sharing banks row-disjoint
                den = pmm.tile([P, N], F32, tag="mm", name="den")
                for tcc in range(TC):
                    for nh in range(2):
                        for hl in range(2):
                            nc.tensor.matmul(
                                den[hl * D:(hl + 1) * D, nh * 512:(nh + 1) * 512],
                                ones_sb[:],
                                pts[hl][:, tcc, nh * 512:(nh + 1) * 512],
                                start=(tcc == 0),
                                stop=(tcc == TC - 1),
                            )
                # raw attention outputs, col-tiled the same way
                otp = pmm.tile([P, N], F32, tag="mm", name="otp")
                for tcc in range(TC):
                    for nh in range(2):
                        for hl in range(2):
                            nc.tensor.matmul(
                                otp[hl * D:(hl + 1) * D, nh * 512:(nh + 1) * 512],
                                vT[:, tcc, 2 * p + hl, :],
                                pts[hl][:, tcc, nh * 512:(nh + 1) * 512],
                                start=(tcc == 0),
                                stop=(tcc == TC - 1),
                            )
                # one full-lane reciprocal + one full-lane multiply per pair
                rec = rpool.tile([P, N], F32, tag="rec")
                nc.vector.reciprocal_approx_fast(rec[:], den[:])
                nc.vector.tensor_mul(oT[:, p, :], otp[:], rec[:])

            # ---- pre-attention: kT, qT jc0-2 — units alternate between
            # the two psum pools (pst is idle until attention) so the
            # copy+sem latency of unit i never stalls unit i+2
            pre = [(kt_chunk, (0,)), (kt_chunk, (1,)),
                   (q_half, (0, 0)), (q_half, (0, 1)),
                   (q_half, (1, 0)), (q_half, (1, 1)),
                   (kt_chunk, (2,)), (kt_chunk, (3,)),
                   (q_half, (2, 0)), (q_half, (2, 1)),
                   (kt_chunk, (4,)), (kt_chunk, (5,))]
            POOLS = [pmm, pst]
            for ui, (fn, args) in enumerate(pre):
                fn(*args, pool=POOLS[ui % 2])

            # ACT exp-table preload: walrus inserts the table load before the
            # first ACTIVATE; keep that off the DMA-issue window
            dume = cpool.tile([P, 8], MDT)
            nc.scalar.activation(dume[:], warm[:, 0:8],
                                 mybir.ActivationFunctionType.Exp)

            # ---- attention: 6 head-pairs, software-pipelined so pair p-1's
            # den/otp matmuls run under pair p's exp windows ----
            pending = None
            for p in range(CC):
                pts = [ptpool.tile([P, TC, N], MDT, tag="pt",
                                   name=f"pt{(2 * p + hl) % 4}")
                       for hl in range(2)]
                st_stage(p, 0, pts)
                if pending is not None:
                    denotp_stage(*pending)
                st_stage(p, 1, pts)
                fill(FILL_PLAN[p])
                pending = (p, pts)
            denotp_stage(*pending)

            # ---- out projection: outT = Wo.T @ oT ----
            # last oc as two separate 1-bank half-tiles so the final
            # copy+DMA tail overlaps the last matmuls
            oqs = [nc.sync, nc.scalar]
            for oc in range(CC - 1):
                ps = pmm.tile([P, N], F32, tag="mm", name=f"o{oc % 2}")
                for jc in range(CC):
                    for nh in range(2):
                        nc.tensor.matmul(
                            ps[:, nh * 512:(nh + 1) * 512],
                            wo_sb[:, jc, oc * P:(oc + 1) * P],
                            oT[:, jc, nh * 512:(nh + 1) * 512],
                            start=(jc == 0),
                            stop=(jc == CC - 1),
                        )
                ob = opool.tile([P, N], F16, tag="ob")
                nc.scalar.copy(ob[:, 0:512], ps[:, 0:512])
                nc.vector.tensor_copy(ob[:, 512:1024], ps[:, 512:1024])
                oqs[oc % 2].dma_start(out_r[:, oc, 0:512], ob[:, 0:512])
                oqs[(oc + 1) % 2].dma_start(
                    out_r[:, oc, 512:1024], ob[:, 512:1024])
            oc = CC - 1
            ob = opool.tile([P, N], F16, tag="ob")
            for nh in range(2):
                ph = pmm.tile([P, 512], F32, tag="mm", name=f"oh{nh}")
                for jc in range(CC):
                    nc.tensor.matmul(
                        ph[:],
                        wo_sb[:, jc, oc * P:(oc + 1) * P],
                        oT[:, jc, nh * 512:(nh + 1) * 512],
                        start=(jc == 0),
                        stop=(jc == CC - 1),
                    )
                sl = slice(nh * 512, (nh + 1) * 512)
                if nh == 0:
                    nc.scalar.copy(ob[:, sl], ph[:])
                else:
                    nc.vector.tensor_copy(ob[:, sl], ph[:])
                oqs[nh].dma_start(out_r[:, oc, sl], ob[:, sl])

    nc.compile()
    _NC_CACHE["nc"] = nc
    return nc


def kernel(x, context, context_mask, Wq, bq, Wk, bk, Wv, bv, Wo, bo,
           _trace=False):
    np_dt = np.float16
    x = np.asarray(x, dtype=np.float32)
    context = np.asarray(context, dtype=np.float32)
    context_mask = np.asarray(context_mask)
    scale = float(D) ** -0.5
    wq = np.ascontiguousarray(np.asarray(Wq) * scale).astype(np_dt)
    # wk permuted to [jc, p, kc*128]: wk_p[j, p, k*128+m] = Wk[k*128+p, j*128+m]
    wk = np.ascontiguousarray(
        np.asarray(Wk).reshape(KC, P, CC, P).transpose(2, 1, 0, 3)
        .reshape(CC, P, CTX)).astype(np_dt)
    wv = np.ascontiguousarray(np.asarray(Wv)).astype(np_dt)
    wo = np.ascontiguousarray(np.asarray(Wo)).astype(np_dt)

    in_maps = []
    for b in range(B):
        mb = np.where(context_mask[b] != 0, 0.0, MASK_NEG).astype(np.float32)
        im = {
            "x": np.ascontiguousarray(x[b].reshape(C, N)).astype(np_dt),
            "ctxT": np.ascontiguousarray(context[b].T).astype(np_dt),
            "mb": np.ascontiguousarray(mb.reshape(TC, P).T),
            "wq": wq, "wk": wk, "wv": wv, "wo": wo,
        }
        in_maps.append(im)

    nc = _build_nc()
    try:
        res = run_bass_kernel_spmd(nc, in_maps, list(range(B)), trace=_trace)
    except Exception:
        # transient NRT_EXEC_UNIT_UNRECOVERABLE etc. — one retry
        res = run_bass_kernel_spmd(nc, in_maps, list(range(B)), trace=_trace)
    out = np.stack([
        res.results[b]["out"].astype(np.float32).reshape(C, 32, 32)
        for b in range(B)
    ])
    if _trace:
        kernel.last_exec_time_ns = res.exec_time_ns
        kernel.last_results = res
    return out


# revision 22
# speedup vs baseline: 1.0943x; 1.0126x over previous
"""CrossAttention2D TRN2 Bass kernel — data-parallel over batch on 8 NeuronCores.

Per core (one batch element), computed fully transposed ("feature-major"):
  qT[j,n]  = Wq'[c,j].T @ x[c,n]            (Wq' = Wq*scale)
  kT[j,t]  = Wk[c,j].T @ ctxT[c,t]
  v[t,j]   = ctxT[c,t].T @ Wv[c,j]
  ST[t,n]  = kT_h[d,t].T @ qT_h[d,n]        (per head, K=64)
  PT[t,n]  = exp(ST + maskbias[t])          (mask folds into ACT per-partition bias)
  OT       = [1(x64) | v_h][t,128].T @ PT[t,n]
             rows 0:64 = softmax denominator replicated, rows 64:128 = raw O
  oT[d,n]  = OT[64:128] * recip_fast(OT[0:64])   (one DVE op each)
  outT[co,n] = Wo[j,co].T @ oT[j,n]

v2 schedule (vs baseline):
  - Head pairs: the two K=64 ST matmuls of heads (2p, 2p+1) are issued
    back-to-back at PE row offsets 0/64 so they genuinely co-execute.
  - Fused exp: one ACTIVATE over [128, 2048] psum (both heads of a pair,
    one tcc) instead of two [128,1024] — cuts ACT fixed cost.
  - qT chunks jc3-5, v half 1, and v half 0 are interleaved INTO the
    attention pair loop as "fill units" so PE chews projection work while
    ACT exps run (attention phase is otherwise ACT-bound).
  - DMA: ctx+wk(jc01) first so kT starts ~10us; x/wq-lo interleaved per-kc
    so qT streams at arrival pace; wv/wq-hi/wo later (JIT deadlines).
  - Copies routed off ACT during attention (DVE), on ACT elsewhere.
  - PSUM: "st" pool 1x[128,2048] (4 banks) + "mm" pool 2x[128,1024] (4 banks).
Output fp16, upcast on host.
"""

import sys

for _p in ("/opt/trn_rl_repo", "/opt/pypackages"):
    if _p not in sys.path:
        sys.path.append(_p)

import numpy as np

import concourse.bass as bass
import concourse.tile as tile
from concourse import bacc, mybir
from concourse.bass_utils import run_bass_kernel_spmd

F32 = mybir.dt.float32
F16 = mybir.dt.float16

B = 8
C = 768            # DIM (q/out features)
CTX = 1024         # CTX_DIM
T = 256            # context tokens
N = 1024           # image tokens (32*32)
NH = 12            # heads
D = 64             # head dim
P = 128
CC = C // P        # 6
TC = T // P        # 2
KC = CTX // P      # 8
MASK_NEG = -30.0

_NC_CACHE = {}


def _build_nc():
    if "nc" in _NC_CACHE:
        return _NC_CACHE["nc"]

    MDT = F16

    nc = bacc.Bacc("TRN2", target_bir_lowering=False, debug=False)

    x_d = nc.dram_tensor("x", [C, N], MDT, kind="ExternalInput")
    ctxT_d = nc.dram_tensor("ctxT", [CTX, T], MDT, kind="ExternalInput")
    mb_d = nc.dram_tensor("mb", [P, TC], F32, kind="ExternalInput")
    wq_d = nc.dram_tensor("wq", [C, C], MDT, kind="ExternalInput")
    # wk host-permuted to [jc, p, kc*128] so each per-jc DMA piece reads
    # 2KB-contiguous DRAM lines per partition
    wk_d = nc.dram_tensor("wk", [CC, P, CTX], MDT, kind="ExternalInput")
    wv_d = nc.dram_tensor("wv", [CTX, C], MDT, kind="ExternalInput")
    wo_d = nc.dram_tensor("wo", [C, C], MDT, kind="ExternalInput")
    out_d = nc.dram_tensor("out", [C, N], F16, kind="ExternalOutput")

    wq_r = wq_d.rearrange("(k p) m -> p k m", p=P)
    x_r = x_d.rearrange("(k p) n -> p k n", p=P)
    ctx_r = ctxT_d.rearrange("(k p) t -> p k t", p=P)
    wk_r = wk_d.rearrange("j p (k m) -> j p k m", m=P)
    wv_r = wv_d.rearrange("(k p) m -> p k m", p=P)
    wo_r = wo_d.rearrange("(k p) m -> p k m", p=P)
    out_r = out_d.rearrange("(k p) n -> p k n", p=P)

    with tile.TileContext(nc) as tc:
        with (
            tc.tile_pool(name="consts", bufs=1) as cpool,
            tc.tile_pool(name="acts", bufs=1) as apool,
            tc.tile_pool(name="s1", bufs=1) as s1pool,
            tc.tile_pool(name="pt", bufs=4) as ptpool,
            tc.tile_pool(name="rec", bufs=4) as rpool,
            tc.tile_pool(name="outsb", bufs=3) as opool,
            tc.tile_pool(name="pst", bufs=2, space="PSUM") as pst,
            tc.tile_pool(name="pmm", bufs=2, space="PSUM") as pmm,
        ):
            # ---- SBUF input tiles ----
            x_sb = s1pool.tile([P, CC, N], MDT)
            wq_sb = s1pool.tile([P, CC, C], MDT)
            ctxT_sb = s1pool.tile([P, KC, T], MDT)
            wk_sb = s1pool.tile([P, CC, KC, P], MDT)   # (p, jc, kc, m)
            wv_sb = s1pool.tile([P, KC, C], MDT)
            wo_sb = s1pool.tile([P, CC, C], MDT)
            mb_sb = cpool.tile([P, TC], F32)

            # ---- DMA issue order (per-queue order = arrival order).
            # sync:   ctx04 wkj0 [x0 wq1lo x2 wq3lo x4 wq5lo] wkj2 wkj4
            #         wv0a wv1a wqhi_a wo_a
            # scalar: ctx48 wkj1 [x1 wq0lo x3 wq2lo x5 wq4lo] wkj3 wkj5
            #         wv0b wv1b wqhi_b wo_b
            H1 = 384   # wq/wv column split: cols 0:384 = jc0-2 / heads 0-5
            wk_pr = wk_d.rearrange("j p (k m) -> p j k m", m=P)
            with tc.high_priority():
                # consolidated pieces; wk j0/j1 first so kT starts earliest
                nc.sync.dma_start(wk_sb[:, 0:1, :, :], wk_pr[:, 0:1])
                nc.scalar.dma_start(wk_sb[:, 1:2, :, :], wk_pr[:, 1:2])
                nc.sync.dma_start(ctxT_sb[:, 0:4, :], ctx_r[:, 0:4, :])
                nc.scalar.dma_start(ctxT_sb[:, 4:8, :], ctx_r[:, 4:8, :])
                nc.sync.dma_start(wk_sb[:, 2:4, :, :], wk_pr[:, 2:4])
                nc.scalar.dma_start(wk_sb[:, 4:6, :, :], wk_pr[:, 4:6])
                nc.scalar.dma_start(wq_sb[:, :, 0:H1], wq_r[:, :, 0:H1])
                nc.sync.dma_start(x_sb[:, 0:2, :], x_r[:, 0:2, :])
                nc.scalar.dma_start(x_sb[:, 2:4, :], x_r[:, 2:4, :])
                nc.sync.dma_start(x_sb[:, 4:6, :], x_r[:, 4:6, :])
                # wv half 0 (heads 0-5), split across queues by kc
                nc.sync.dma_start(wv_sb[:, 0:4, 0:H1], wv_r[:, 0:4, 0:H1])
                nc.scalar.dma_start(wv_sb[:, 4:8, 0:H1], wv_r[:, 4:8, 0:H1])
                # wv half 1 (heads 6-11)
                nc.sync.dma_start(wv_sb[:, 0:4, H1:C], wv_r[:, 0:4, H1:C])
                nc.scalar.dma_start(wv_sb[:, 4:8, H1:C], wv_r[:, 4:8, H1:C])
                # wq-hi (jc3-5)
                nc.sync.dma_start(wq_sb[:, :, H1:C], wq_r[:, :, H1:C])
                # wo
                nc.scalar.dma_start(wo_sb[:, :, 0:H1], wo_r[:, :, 0:H1])
                nc.sync.dma_start(wo_sb[:, :, H1:C], wo_r[:, :, H1:C])
                nc.gpsimd.dma_start(mb_sb[:], mb_d[:])

            # ---- activation tiles ----
            qT = apool.tile([P, CC, N], MDT)
            kT = apool.tile([P, CC, T], MDT)
            vT = apool.tile([P, TC, NH, D], MDT)
            oT = apool.tile([P, CC, N], MDT)
            ones_sb = cpool.tile([P, D], MDT)
            nc.gpsimd.memset(ones_sb[:], 1.0)

            # ---- PE warm-up junk ----
            warm = cpool.tile([P, 256], MDT)
            nc.vector.memset(warm[:], 0.0)
            for w in range(24):
                wps = pmm.tile([P, N], F32, tag="mm", name=f"warm{w % 4}")
                nc.tensor.matmul(wps[:, 0:256], warm[:, 0:P], warm[:],
                                 start=True, stop=True)

            # ---- stage helpers (self-contained psum units) ----
            def ptile(pool, name):
                return pool.tile([P, N], F32,
                                 tag="mm" if pool is pmm else "st", name=name)

            def kt_chunk(jc, pool=None):
                ps = ptile(pool or pmm, f"k{jc}")
                for kc in range(KC):
                    nc.tensor.matmul(
                        ps[:, 0:T],
                        wk_sb[:, jc, kc, :],
                        ctxT_sb[:, kc, :],
                        start=(kc == 0),
                        stop=(kc == KC - 1),
                    )
                nc.vector.tensor_copy(kT[:, jc, :], ps[:, 0:T])

            def q_mms(ps, jc, nh, kcs):
                for kc in kcs:
                    nc.tensor.matmul(
                        ps[:, 0:512],
                        wq_sb[:, kc, jc * P:(jc + 1) * P],
                        x_sb[:, kc, nh * 512:(nh + 1) * 512],
                        start=(kc == 0),
                        stop=(kc == CC - 1),
                    )

            def q_half(jc, nh, pool=None):
                # qT[:, jc, nh*512:(nh+1)*512], 6 kc-accumulating mms
                ps = ptile(pool or pmm, f"q{jc}_{nh}")
                q_mms(ps, jc, nh, range(CC))
                nc.vector.tensor_copy(qT[:, jc, nh * 512:(nh + 1) * 512],
                                      ps[:, 0:512])

            def v_unit(jh, tcc, pool=None):
                # heads 6jh..6jh+5, one t-chunk
                ps = ptile(pool or pmm, f"v{jh}_{tcc}")
                for kc in range(KC):
                    nc.tensor.matmul(
                        ps[:, 0:H1],
                        ctxT_sb[:, kc, tcc * P:(tcc + 1) * P],
                        wv_sb[:, kc, jh * H1:(jh + 1) * H1],
                        start=(kc == 0),
                        stop=(kc == KC - 1),
                    )
                nc.vector.tensor_copy(
                    vT[:, tcc, 6 * jh:6 * jh + 6, :],
                    ps[:, 0:H1].rearrange("p (h d) -> p h d", d=D),
                )

            # fill units consumed inside the attention pair loop
            fills = [
                lambda: v_unit(0, 0),
                lambda: v_unit(0, 1),
                lambda: q_half(3, 0),
                lambda: q_half(3, 1),
                lambda: v_unit(1, 0),
                lambda: v_unit(1, 1),
                lambda: q_half(4, 0),
                lambda: q_half(4, 1),
                lambda: q_half(5, 0),
                lambda: q_half(5, 1),
            ]
            FILL_PLAN = [2, 2, 2, 2, 2, 0]   # units consumed per pair
            fi = [0]

            def fill(k=1):
                for _ in range(k):
                    if fi[0] < len(fills):
                        fills[fi[0]]()
                        fi[0] += 1

            def st_stage(p, tcc, pts):
                # per-head score tiles; the two heads' K=64 mms issue
                # adjacently at PE row offsets 0/64 so they co-execute
                sts = [pst.tile([P, N], F32, tag="st", name=f"st{hl}")
                       for hl in range(2)]
                for nh in range(2):
                    for hl in range(2):
                        r0 = hl * D
                        nc.tensor.matmul(
                            sts[hl][:, nh * 512:(nh + 1) * 512],
                            kT[r0:r0 + D, p, tcc * P:(tcc + 1) * P],
                            qT[r0:r0 + D, p, nh * 512:(nh + 1) * 512],
                            start=True,
                            stop=True,
                        )
                for hl in range(2):
                    nc.scalar.activation(
                        pts[hl][:, tcc, :],
                        sts[hl][:],
                        mybir.ActivationFunctionType.Exp,
                        bias=mb_sb[:, tcc:tcc + 1],
                    )

            def denotp_stage(p, pts):
                # denominators: den[hl*64:...] = ones.T @ PT_hl — col-tiled
                # pair (positions (0,0)/(0,64)) sharing banks row-disjoint
                den = pmm.tile([P, N], F32, tag="mm", name="den")
                for tcc in range(TC):
                    for nh in range(2):
                        for hl in range(2):
                            nc.tensor.matmul(
                                den[hl * D:(hl + 1) * D, nh * 512:(nh + 1) * 512],
                                ones_sb[:],
                                pts[hl][:, tcc, nh * 512:(nh + 1) * 512],
                                start=(tcc == 0),
                                stop=(tcc == TC - 1),
                            )
                # raw attention outputs, col-tiled the same way
                otp = pmm.tile([P, N], F32, tag="mm", name="otp")
                for tcc in range(TC):
                    for nh in range(2):
                        for hl in range(2):
                            nc.tensor.matmul(
                                otp[hl * D:(hl + 1) * D, nh * 512:(nh + 1) * 512],
                                vT[:, tcc, 2 * p + hl, :],
                                pts[hl][:, tcc, nh * 512:(nh + 1) * 512],
                                start=(tcc == 0),
                                stop=(tcc == TC - 1),
                            )
                # one full-lane reciprocal + one full-lane multiply per pair
                rec = rpool.tile([P, N], F32, tag="rec")
                nc.vector.reciprocal_approx_fast(rec[:], den[:])
                nc.vector.tensor_mul(oT[:, p, :], otp[:], rec[:])

            # ---- pre-attention: kT, qT jc0-2 — units alternate between
            # the two psum pools (pst is idle until attention) so the
            # copy+sem latency of unit i never stalls unit i+2
            POOLS = [pmm, pst]
            for jc in range(CC):
                kt_chunk(jc, pool=POOLS[jc % 2])
            # q jc0/jc1: four tiles, kc0-2 mms first (x kc0-2 lands
            # earlier), then kc3-5 + copies — keeps PE fed while x streams
            qtiles = {}
            for ui, (jc, nh) in enumerate([(0, 0), (0, 1), (1, 0), (1, 1)]):
                ps = ptile(POOLS[ui % 2], f"q{jc}_{nh}")
                qtiles[(jc, nh)] = ps
                q_mms(ps, jc, nh, range(3))
            for ui, (jc, nh) in enumerate([(0, 0), (0, 1), (1, 0), (1, 1)]):
                ps = qtiles[(jc, nh)]
                q_mms(ps, jc, nh, range(3, CC))
                nc.vector.tensor_copy(qT[:, jc, nh * 512:(nh + 1) * 512],
                                      ps[:, 0:512])
            q_half(2, 0, pool=pmm)
            q_half(2, 1, pool=pst)

            # ACT exp-table preload: walrus inserts the table load before the
            # first ACTIVATE; keep that off the DMA-issue window
            dume = cpool.tile([P, 8], MDT)
            nc.scalar.activation(dume[:], warm[:, 0:8],
                                 mybir.ActivationFunctionType.Exp)

            # ---- attention: 6 head-pairs, software-pipelined so pair p-1's
            # den/otp matmuls run under pair p's exp windows ----
            pending = None
            for p in range(CC):
                pts = [ptpool.tile([P, TC, N], MDT, tag="pt",
                                   name=f"pt{(2 * p + hl) % 4}")
                       for hl in range(2)]
                st_stage(p, 0, pts)
                if pending is not None:
                    denotp_stage(*pending)
                st_stage(p, 1, pts)
                fill(FILL_PLAN[p])
                pending = (p, pts)
            denotp_stage(*pending)

            # ---- out projection: outT = Wo.T @ oT ----
            # last oc as two separate 1-bank half-tiles so the final
            # copy+DMA tail overlaps the last matmuls
            oqs = [nc.sync, nc.scalar]
            for oc in range(CC - 1):
                ps = pmm.tile([P, N], F32, tag="mm", name=f"o{oc % 2}")
                for jc in range(CC):
                    for nh in range(2):
                        nc.tensor.matmul(
                            ps[:, nh * 512:(nh + 1) * 512],
                            wo_sb[:, jc, oc * P:(oc + 1) * P],
                            oT[:, jc, nh * 512:(nh + 1) * 512],
                            start=(jc == 0),
                            stop=(jc == CC - 1),
                        )
                ob = opool.tile([P, N], F16, tag="ob")
                nc.scalar.copy(ob[:, 0:512], ps[:, 0:512])
                nc.vector.tensor_copy(ob[:, 512:1024], ps[:, 512:1024])
                oqs[oc % 2].dma_start(out_r[:, oc, 0:512], ob[:, 0:512])
                oqs[(oc + 1) % 2].dma_start(
                    out_r[:, oc, 512:1024], ob[:, 512:1024])
            oc = CC - 1
            ob = opool.tile([P, N], F16, tag="ob")
            for nh in range(2):
                ph = pmm.tile([P, 512], F32, tag="mm", name=f"oh{nh}")
                for jc in range(CC):
                    nc.tensor.matmul(
                        ph[:],
                        wo_sb[:, jc, oc * P:(oc + 1) * P],
                        oT[:, jc, nh * 512:(nh + 1) * 512],
                        start=(jc == 0),
                        stop=(jc == CC - 1),
                    )
                sl = slice(nh * 512, (nh + 1) * 512)
                if nh == 0:
                    nc.scalar.copy(ob[:, sl], ph[:])
                else:
                    nc.vector.tensor_copy(ob[:, sl], ph[:])
                oqs[nh].dma_start(out_r[:, oc, sl], ob[:, sl])

    nc.compile()
    _NC_CACHE["nc"] = nc
    return nc


def kernel(x, context, context_mask, Wq, bq, Wk, bk, Wv, bv, Wo, bo,
           _trace=False):
    np_dt = np.float16
    x = np.asarray(x, dtype=np.float32)
    context = np.asarray(context, dtype=np.float32)
    context_mask = np.asarray(context_mask)
    scale = float(D) ** -0.5
    wq = np.ascontiguousarray(np.asarray(Wq) * scale).astype(np_dt)
    # wk permuted to [jc, p, kc*128]: wk_p[j, p, k*128+m] = Wk[k*128+p, j*128+m]
    wk = np.ascontiguousarray(
        np.asarray(Wk).reshape(KC, P, CC, P).transpose(2, 1, 0, 3)
        .reshape(CC, P, CTX)).astype(np_dt)
    wv = np.ascontiguousarray(np.asarray(Wv)).astype(np_dt)
    wo = np.ascontiguousarray(np.asarray(Wo)).astype(np_dt)

    in_maps = []
    for b in range(B):
        mb = np.where(context_mask[b] != 0, 0.0, MASK_NEG).astype(np.float32)
        im = {
            "x": np.ascontiguousarray(x[b].reshape(C, N)).astype(np_dt),
            "ctxT": np.ascontiguousarray(context[b].T).astype(np_dt),
            "mb": np.ascontiguousarray(mb.reshape(TC, P).T),
            "wq": wq, "wk": wk, "wv": wv, "wo": wo,
        }
        in_maps.append(im)

    nc = _build_nc()
    try:
        res = run_bass_kernel_spmd(nc, in_maps, list(range(B)), trace=_trace)
    except Exception:
        # transient NRT_EXEC_UNIT_UNRECOVERABLE etc. — one retry
        res = run_bass_kernel_spmd(nc, in_maps, list(range(B)), trace=_trace)
    out = np.stack([
        res.results[b]["out"].astype(np.float32).reshape(C, 32, 32)
        for b in range(B)
    ])
    if _trace:
        kernel.last_exec_time_ns = res.exec_time_ns
        kernel.last_results = res
    return out


# revision 23
# speedup vs baseline: 1.1101x; 1.0144x over previous
"""CrossAttention2D TRN2 Bass kernel — data-parallel over batch on 8 NeuronCores.

Per core (one batch element), computed fully transposed ("feature-major"):
  qT[j,n]  = Wq'[c,j].T @ x[c,n]            (Wq' = Wq*scale)
  kT[j,t]  = Wk[c,j].T @ ctxT[c,t]
  v[t,j]   = ctxT[c,t].T @ Wv[c,j]
  ST[t,n]  = kT_h[d,t].T @ qT_h[d,n]        (per head, K=64)
  PT[t,n]  = exp(ST + maskbias[t])          (mask folds into ACT per-partition bias)
  OT       = [1(x64) | v_h][t,128].T @ PT[t,n]
             rows 0:64 = softmax denominator replicated, rows 64:128 = raw O
  oT[d,n]  = OT[64:128] * recip_fast(OT[0:64])   (one DVE op each)
  outT[co,n] = Wo[j,co].T @ oT[j,n]

v2 schedule (vs baseline):
  - Head pairs: the two K=64 ST matmuls of heads (2p, 2p+1) are issued
    back-to-back at PE row offsets 0/64 so they genuinely co-execute.
  - Fused exp: one ACTIVATE over [128, 2048] psum (both heads of a pair,
    one tcc) instead of two [128,1024] — cuts ACT fixed cost.
  - qT chunks jc3-5, v half 1, and v half 0 are interleaved INTO the
    attention pair loop as "fill units" so PE chews projection work while
    ACT exps run (attention phase is otherwise ACT-bound).
  - DMA: ctx+wk(jc01) first so kT starts ~10us; x/wq-lo interleaved per-kc
    so qT streams at arrival pace; wv/wq-hi/wo later (JIT deadlines).
  - Copies routed off ACT during attention (DVE), on ACT elsewhere.
  - PSUM: "st" pool 1x[128,2048] (4 banks) + "mm" pool 2x[128,1024] (4 banks).
Output fp16, upcast on host.
"""

import sys

for _p in ("/opt/trn_rl_repo", "/opt/pypackages"):
    if _p not in sys.path:
        sys.path.append(_p)

import numpy as np

import concourse.bass as bass
import concourse.tile as tile
from concourse import bacc, mybir
from concourse.bass_utils import run_bass_kernel_spmd

F32 = mybir.dt.float32
F16 = mybir.dt.float16

B = 8
C = 768            # DIM (q/out features)
CTX = 1024         # CTX_DIM
T = 256            # context tokens
N = 1024           # image tokens (32*32)
NH = 12            # heads
D = 64             # head dim
P = 128
CC = C // P        # 6
TC = T // P        # 2
KC = CTX // P      # 8
MASK_NEG = -30.0

_NC_CACHE = {}


def _build_nc():
    if "nc" in _NC_CACHE:
        return _NC_CACHE["nc"]

    MDT = F16

    nc = bacc.Bacc("TRN2", target_bir_lowering=False, debug=False)

    # ALL inputs host-packed partition-major so every DMA piece is a
    # contiguous >=2KB run per partition on both DRAM and SBUF sides
    x_d = nc.dram_tensor("x", [P, CC, N], MDT, kind="ExternalInput")
    ctxT_d = nc.dram_tensor("ctxT", [P, KC, T], MDT, kind="ExternalInput")
    mb_d = nc.dram_tensor("mb", [P, TC], F32, kind="ExternalInput")
    H1 = 384
    wql_d = nc.dram_tensor("wql", [P, CC, H1], MDT, kind="ExternalInput")
    wqh_d = nc.dram_tensor("wqh", [P, CC, H1], MDT, kind="ExternalInput")
    wk_d = nc.dram_tensor("wk", [CC, P, CTX], MDT, kind="ExternalInput")
    wvl_d = nc.dram_tensor("wvl", [P, KC, H1], MDT, kind="ExternalInput")
    wvh_d = nc.dram_tensor("wvh", [P, KC, H1], MDT, kind="ExternalInput")
    wol_d = nc.dram_tensor("wol", [P, CC, H1], MDT, kind="ExternalInput")
    woh_d = nc.dram_tensor("woh", [P, CC, H1], MDT, kind="ExternalInput")
    out_d = nc.dram_tensor("out", [C, N], F16, kind="ExternalOutput")

    wk_r = wk_d.rearrange("j p (k m) -> j p k m", m=P)
    out_r = out_d.rearrange("(k p) n -> p k n", p=P)

    with tile.TileContext(nc) as tc:
        with (
            tc.tile_pool(name="consts", bufs=1) as cpool,
            tc.tile_pool(name="acts", bufs=1) as apool,
            tc.tile_pool(name="s1", bufs=1) as s1pool,
            tc.tile_pool(name="pt", bufs=4) as ptpool,
            tc.tile_pool(name="rec", bufs=4) as rpool,
            tc.tile_pool(name="outsb", bufs=3) as opool,
            tc.tile_pool(name="pst", bufs=2, space="PSUM") as pst,
            tc.tile_pool(name="pmm", bufs=2, space="PSUM") as pmm,
        ):
            # ---- SBUF input tiles ----
            x_sb = s1pool.tile([P, CC, N], MDT)
            wql_sb = s1pool.tile([P, CC, H1], MDT)
            wqh_sb = s1pool.tile([P, CC, H1], MDT)
            ctxT_sb = s1pool.tile([P, KC, T], MDT)
            wk_sb = s1pool.tile([P, CC, KC, P], MDT)   # (p, jc, kc, m)
            wvl_sb = s1pool.tile([P, KC, H1], MDT)
            wvh_sb = s1pool.tile([P, KC, H1], MDT)
            wol_sb = s1pool.tile([P, CC, H1], MDT)
            woh_sb = s1pool.tile([P, CC, H1], MDT)
            mb_sb = cpool.tile([P, TC], F32)

            # ---- DMA issue order (per-queue order = arrival order;
            # pieces ordered by PE consumption: kT, qT-a, qT-b, fills, out)
            wk_pr = wk_d.rearrange("j p (k m) -> p j k m", m=P)
            with tc.high_priority():
                nc.sync.dma_start(wk_sb[:, 0:1, :, :], wk_pr[:, 0:1])
                nc.scalar.dma_start(wk_sb[:, 1:2, :, :], wk_pr[:, 1:2])
                nc.sync.dma_start(ctxT_sb[:, 0:4, :], ctxT_d[:, 0:4, :])
                nc.scalar.dma_start(ctxT_sb[:, 4:8, :], ctxT_d[:, 4:8, :])
                nc.sync.dma_start(wk_sb[:, 2:4, :, :], wk_pr[:, 2:4])
                nc.scalar.dma_start(wk_sb[:, 4:6, :, :], wk_pr[:, 4:6])
                nc.sync.dma_start(x_sb[:, 0:2, :], x_d[:, 0:2, :])
                nc.scalar.dma_start(wql_sb[:], wql_d[:])
                nc.scalar.dma_start(x_sb[:, 2:4, :], x_d[:, 2:4, :])
                nc.sync.dma_start(x_sb[:, 4:6, :], x_d[:, 4:6, :])
                nc.sync.dma_start(wvl_sb[:], wvl_d[:])
                nc.scalar.dma_start(wvh_sb[:], wvh_d[:])
                nc.sync.dma_start(wqh_sb[:], wqh_d[:])
                nc.scalar.dma_start(woh_sb[:], woh_d[:])
                nc.sync.dma_start(wol_sb[:], wol_d[:])
                nc.gpsimd.dma_start(mb_sb[:], mb_d[:])

            # ---- activation tiles ----
            qT = apool.tile([P, CC, N], MDT)
            kT = apool.tile([P, CC, T], MDT)
            vT = apool.tile([P, TC, NH, D], MDT)
            oT = apool.tile([P, CC, N], MDT)
            ones_sb = cpool.tile([P, D], MDT)
            nc.gpsimd.memset(ones_sb[:], 1.0)

            # ---- PE warm-up junk ----
            warm = cpool.tile([P, 256], MDT)
            nc.vector.memset(warm[:], 0.0)
            for w in range(10):
                wps = pmm.tile([P, N], F32, tag="mm", name=f"warm{w % 4}")
                nc.tensor.matmul(wps[:, 0:256], warm[:, 0:P], warm[:],
                                 start=True, stop=True)

            # ---- stage helpers (self-contained psum units) ----
            def ptile(pool, name):
                return pool.tile([P, N], F32,
                                 tag="mm" if pool is pmm else "st", name=name)

            def kt_chunk(jc, pool=None):
                ps = ptile(pool or pmm, f"k{jc}")
                for kc in range(KC):
                    nc.tensor.matmul(
                        ps[:, 0:T],
                        wk_sb[:, jc, kc, :],
                        ctxT_sb[:, kc, :],
                        start=(kc == 0),
                        stop=(kc == KC - 1),
                    )
                nc.vector.tensor_copy(kT[:, jc, :], ps[:, 0:T])

            def q_mms(ps, jc, nh, kcs):
                wq_sb = wql_sb if jc < 3 else wqh_sb
                jcl = jc % 3
                for kc in kcs:
                    nc.tensor.matmul(
                        ps[:, 0:512],
                        wq_sb[:, kc, jcl * P:(jcl + 1) * P],
                        x_sb[:, kc, nh * 512:(nh + 1) * 512],
                        start=(kc == 0),
                        stop=(kc == CC - 1),
                    )

            def q_half(jc, nh, pool=None):
                # qT[:, jc, nh*512:(nh+1)*512], 6 kc-accumulating mms
                ps = ptile(pool or pmm, f"q{jc}_{nh}")
                q_mms(ps, jc, nh, range(CC))
                nc.vector.tensor_copy(qT[:, jc, nh * 512:(nh + 1) * 512],
                                      ps[:, 0:512])

            def v_unit(jh, tcc, pool=None):
                # heads 6jh..6jh+5, one t-chunk
                ps = ptile(pool or pmm, f"v{jh}_{tcc}")
                for kc in range(KC):
                    nc.tensor.matmul(
                        ps[:, 0:H1],
                        ctxT_sb[:, kc, tcc * P:(tcc + 1) * P],
                        (wvl_sb if jh == 0 else wvh_sb)[:, kc, :],
                        start=(kc == 0),
                        stop=(kc == KC - 1),
                    )
                nc.vector.tensor_copy(
                    vT[:, tcc, 6 * jh:6 * jh + 6, :],
                    ps[:, 0:H1].rearrange("p (h d) -> p h d", d=D),
                )

            # fill units consumed inside the attention pair loop
            fills = [
                lambda: v_unit(0, 0),
                lambda: v_unit(0, 1),
                lambda: q_half(3, 0),
                lambda: q_half(3, 1),
                lambda: v_unit(1, 0),
                lambda: v_unit(1, 1),
                lambda: q_half(4, 0),
                lambda: q_half(4, 1),
                lambda: q_half(5, 0),
                lambda: q_half(5, 1),
            ]
            FILL_PLAN = [2, 2, 2, 2, 2, 0]   # units consumed per pair
            fi = [0]

            def fill(k=1):
                for _ in range(k):
                    if fi[0] < len(fills):
                        fills[fi[0]]()
                        fi[0] += 1

            def st_stage(p, tcc, pts):
                # per-head score tiles; the two heads' K=64 mms issue
                # adjacently at PE row offsets 0/64 so they co-execute
                sts = [pst.tile([P, N], F32, tag="st", name=f"st{hl}")
                       for hl in range(2)]
                for nh in range(2):
                    for hl in range(2):
                        r0 = hl * D
                        nc.tensor.matmul(
                            sts[hl][:, nh * 512:(nh + 1) * 512],
                            kT[r0:r0 + D, p, tcc * P:(tcc + 1) * P],
                            qT[r0:r0 + D, p, nh * 512:(nh + 1) * 512],
                            start=True,
                            stop=True,
                        )
                for hl in range(2):
                    nc.scalar.activation(
                        pts[hl][:, tcc, :],
                        sts[hl][:],
                        mybir.ActivationFunctionType.Exp,
                        bias=mb_sb[:, tcc:tcc + 1],
                    )

            def denotp_stage(p, pts):
                # denominators: den[hl*64:...] = ones.T @ PT_hl — col-tiled
                # pair (positions (0,0)/(0,64)) sharing banks row-disjoint
                den = pmm.tile([P, N], F32, tag="mm", name="den")
                for tcc in range(TC):
                    for nh in range(2):
                        for hl in range(2):
                            nc.tensor.matmul(
                                den[hl * D:(hl + 1) * D, nh * 512:(nh + 1) * 512],
                                ones_sb[:],
                                pts[hl][:, tcc, nh * 512:(nh + 1) * 512],
                                start=(tcc == 0),
                                stop=(tcc == TC - 1),
                            )
                # raw attention outputs, col-tiled the same way
                otp = pmm.tile([P, N], F32, tag="mm", name="otp")
                for tcc in range(TC):
                    for nh in range(2):
                        for hl in range(2):
                            nc.tensor.matmul(
                                otp[hl * D:(hl + 1) * D, nh * 512:(nh + 1) * 512],
                                vT[:, tcc, 2 * p + hl, :],
                                pts[hl][:, tcc, nh * 512:(nh + 1) * 512],
                                start=(tcc == 0),
                                stop=(tcc == TC - 1),
                            )
                # one full-lane reciprocal + one full-lane multiply per pair
                rec = rpool.tile([P, N], F32, tag="rec")
                nc.vector.reciprocal_approx_fast(rec[:], den[:])
                nc.vector.tensor_mul(oT[:, p, :], otp[:], rec[:])

            # ---- pre-attention: kT, qT jc0-2 — units alternate between
            # the two psum pools (pst is idle until attention) so the
            # copy+sem latency of unit i never stalls unit i+2
            POOLS = [pmm, pst]
            for jc in range(CC):
                kt_chunk(jc, pool=POOLS[jc % 2])
            # q jc0/jc1: four tiles, kc0-2 mms first (x kc0-2 lands
            # earlier), then kc3-5 + copies — keeps PE fed while x streams
            qtiles = {}
            for ui, (jc, nh) in enumerate([(0, 0), (0, 1), (1, 0), (1, 1)]):
                ps = ptile(POOLS[ui % 2], f"q{jc}_{nh}")
                qtiles[(jc, nh)] = ps
                q_mms(ps, jc, nh, range(3))
            for ui, (jc, nh) in enumerate([(0, 0), (0, 1), (1, 0), (1, 1)]):
                ps = qtiles[(jc, nh)]
                q_mms(ps, jc, nh, range(3, CC))
                nc.vector.tensor_copy(qT[:, jc, nh * 512:(nh + 1) * 512],
                                      ps[:, 0:512])
            q_half(2, 0, pool=pmm)
            q_half(2, 1, pool=pst)

            # ACT exp-table preload: walrus inserts the table load before the
            # first ACTIVATE; keep that off the DMA-issue window
            dume = cpool.tile([P, 8], MDT)
            nc.scalar.activation(dume[:], warm[:, 0:8],
                                 mybir.ActivationFunctionType.Exp)

            # ---- attention: 6 head-pairs, software-pipelined so pair p-1's
            # den/otp matmuls run under pair p's exp windows ----
            pending = None
            for p in range(CC):
                pts = [ptpool.tile([P, TC, N], MDT, tag="pt",
                                   name=f"pt{(2 * p + hl) % 4}")
                       for hl in range(2)]
                st_stage(p, 0, pts)
                if pending is not None:
                    denotp_stage(*pending)
                st_stage(p, 1, pts)
                fill(FILL_PLAN[p])
                pending = (p, pts)
            denotp_stage(*pending)

            # ---- out projection: outT = Wo.T @ oT ----
            # last oc as two separate 1-bank half-tiles so the final
            # copy+DMA tail overlaps the last matmuls
            oqs = [nc.sync, nc.scalar]
            for oc in range(CC - 1):
                ps = pmm.tile([P, N], F32, tag="mm", name=f"o{oc % 2}")
                for jc in range(CC):
                    for nh in range(2):
                        nc.tensor.matmul(
                            ps[:, nh * 512:(nh + 1) * 512],
                            (wol_sb if oc < 3 else woh_sb)[
                                :, jc, (oc % 3) * P:(oc % 3 + 1) * P],
                            oT[:, jc, nh * 512:(nh + 1) * 512],
                            start=(jc == 0),
                            stop=(jc == CC - 1),
                        )
                ob = opool.tile([P, N], F16, tag="ob")
                nc.scalar.copy(ob[:, 0:512], ps[:, 0:512])
                nc.vector.tensor_copy(ob[:, 512:1024], ps[:, 512:1024])
                oqs[oc % 2].dma_start(out_r[:, oc, 0:512], ob[:, 0:512])
                oqs[(oc + 1) % 2].dma_start(
                    out_r[:, oc, 512:1024], ob[:, 512:1024])
            oc = CC - 1
            ob = opool.tile([P, N], F16, tag="ob")
            for nh in range(2):
                ph = pmm.tile([P, 512], F32, tag="mm", name=f"oh{nh}")
                for jc in range(CC):
                    nc.tensor.matmul(
                        ph[:],
                        woh_sb[:, jc, (oc % 3) * P:(oc % 3 + 1) * P],
                        oT[:, jc, nh * 512:(nh + 1) * 512],
                        start=(jc == 0),
                        stop=(jc == CC - 1),
                    )
                sl = slice(nh * 512, (nh + 1) * 512)
                if nh == 0:
                    nc.scalar.copy(ob[:, sl], ph[:])
                else:
                    nc.vector.tensor_copy(ob[:, sl], ph[:])
                oqs[nh].dma_start(out_r[:, oc, sl], ob[:, sl])

    nc.compile()
    _NC_CACHE["nc"] = nc
    return nc


def kernel(x, context, context_mask, Wq, bq, Wk, bk, Wv, bv, Wo, bo,
           _trace=False):
    np_dt = np.float16
    x = np.asarray(x, dtype=np.float32)
    context = np.asarray(context, dtype=np.float32)
    context_mask = np.asarray(context_mask)
    scale = float(D) ** -0.5

    def pk(w, rows):     # [rows*128, 768] -> [128, rows, 768] partition-major
        return np.asarray(w).reshape(rows, P, C).transpose(1, 0, 2)

    wq_p = pk(np.asarray(Wq) * scale, CC)
    wql = np.ascontiguousarray(wq_p[:, :, 0:384]).astype(np_dt)
    wqh = np.ascontiguousarray(wq_p[:, :, 384:768]).astype(np_dt)
    # wk permuted to [jc, p, kc*128]: wk_p[j, p, k*128+m] = Wk[k*128+p, j*128+m]
    wk = np.ascontiguousarray(
        np.asarray(Wk).reshape(KC, P, CC, P).transpose(2, 1, 0, 3)
        .reshape(CC, P, CTX)).astype(np_dt)
    wv_p = pk(Wv, KC)
    wvl = np.ascontiguousarray(wv_p[:, :, 0:384]).astype(np_dt)
    wvh = np.ascontiguousarray(wv_p[:, :, 384:768]).astype(np_dt)
    wo_p = pk(Wo, CC)
    wol = np.ascontiguousarray(wo_p[:, :, 0:384]).astype(np_dt)
    woh = np.ascontiguousarray(wo_p[:, :, 384:768]).astype(np_dt)

    in_maps = []
    for b in range(B):
        mb = np.where(context_mask[b] != 0, 0.0, MASK_NEG).astype(np.float32)
        im = {
            "x": np.ascontiguousarray(
                x[b].reshape(CC, P, N).transpose(1, 0, 2)).astype(np_dt),
            "ctxT": np.ascontiguousarray(
                context[b].T.reshape(KC, P, T).transpose(1, 0, 2)
            ).astype(np_dt),
            "mb": np.ascontiguousarray(mb.reshape(TC, P).T),
            "wql": wql, "wqh": wqh, "wk": wk,
            "wvl": wvl, "wvh": wvh, "wol": wol, "woh": woh,
        }
        in_maps.append(im)

    nc = _build_nc()
    try:
        res = run_bass_kernel_spmd(nc, in_maps, list(range(B)), trace=_trace)
    except Exception:
        # transient NRT_EXEC_UNIT_UNRECOVERABLE etc. — one retry
        res = run_bass_kernel_spmd(nc, in_maps, list(range(B)), trace=_trace)
    out = np.stack([
        res.results[b]["out"].astype(np.float32).reshape(C, 32, 32)
        for b in range(B)
    ])
    if _trace:
        kernel.last_exec_time_ns = res.exec_time_ns
        kernel.last_results = res
    return out


# revision 24
# speedup vs baseline: 1.1130x; 1.0026x over previous
"""CrossAttention2D TRN2 Bass kernel — data-parallel over batch on 8 NeuronCores.

Per core (one batch element), computed fully transposed ("feature-major"):
  qT[j,n]  = Wq'[c,j].T @ x[c,n]            (Wq' = Wq*scale)
  kT[j,t]  = Wk[c,j].T @ ctxT[c,t]
  v[t,j]   = ctxT[c,t].T @ Wv[c,j]
  ST[t,n]  = kT_h[d,t].T @ qT_h[d,n]        (per head, K=64)
  PT[t,n]  = exp(ST + maskbias[t])          (mask folds into ACT per-partition bias)
  OT       = [1(x64) | v_h][t,128].T @ PT[t,n]
             rows 0:64 = softmax denominator replicated, rows 64:128 = raw O
  oT[d,n]  = OT[64:128] * recip_fast(OT[0:64])   (one DVE op each)
  outT[co,n] = Wo[j,co].T @ oT[j,n]

v2 schedule (vs baseline):
  - Head pairs: the two K=64 ST matmuls of heads (2p, 2p+1) are issued
    back-to-back at PE row offsets 0/64 so they genuinely co-execute.
  - Fused exp: one ACTIVATE over [128, 2048] psum (both heads of a pair,
    one tcc) instead of two [128,1024] — cuts ACT fixed cost.
  - qT chunks jc3-5, v half 1, and v half 0 are interleaved INTO the
    attention pair loop as "fill units" so PE chews projection work while
    ACT exps run (attention phase is otherwise ACT-bound).
  - DMA: ctx+wk(jc01) first so kT starts ~10us; x/wq-lo interleaved per-kc
    so qT streams at arrival pace; wv/wq-hi/wo later (JIT deadlines).
  - Copies routed off ACT during attention (DVE), on ACT elsewhere.
  - PSUM: "st" pool 1x[128,2048] (4 banks) + "mm" pool 2x[128,1024] (4 banks).
Output fp16, upcast on host.
"""

import sys

for _p in ("/opt/trn_rl_repo", "/opt/pypackages"):
    if _p not in sys.path:
        sys.path.append(_p)

import numpy as np

import concourse.bass as bass
import concourse.tile as tile
from concourse import bacc, mybir
from concourse.bass_utils import run_bass_kernel_spmd

F32 = mybir.dt.float32
F16 = mybir.dt.float16

B = 8
C = 768            # DIM (q/out features)
CTX = 1024         # CTX_DIM
T = 256            # context tokens
N = 1024           # image tokens (32*32)
NH = 12            # heads
D = 64             # head dim
P = 128
CC = C // P        # 6
TC = T // P        # 2
KC = CTX // P      # 8
MASK_NEG = -30.0

_NC_CACHE = {}


def _build_nc():
    if "nc" in _NC_CACHE:
        return _NC_CACHE["nc"]

    MDT = F16

    nc = bacc.Bacc("TRN2", target_bir_lowering=False, debug=False)

    # ALL inputs host-packed partition-major so every DMA piece is a
    # contiguous >=2KB run per partition on both DRAM and SBUF sides
    x_d = nc.dram_tensor("x", [P, CC, N], MDT, kind="ExternalInput")
    ctxT_d = nc.dram_tensor("ctxT", [P, KC, T], MDT, kind="ExternalInput")
    mb_d = nc.dram_tensor("mb", [P, TC], F32, kind="ExternalInput")
    H1 = 384
    wql_d = nc.dram_tensor("wql", [P, CC, H1], MDT, kind="ExternalInput")
    wqh_d = nc.dram_tensor("wqh", [P, CC, H1], MDT, kind="ExternalInput")
    wk_d = nc.dram_tensor("wk", [CC, P, CTX], MDT, kind="ExternalInput")
    wvl_d = nc.dram_tensor("wvl", [P, KC, H1], MDT, kind="ExternalInput")
    wvh_d = nc.dram_tensor("wvh", [P, KC, H1], MDT, kind="ExternalInput")
    wol_d = nc.dram_tensor("wol", [P, CC, H1], MDT, kind="ExternalInput")
    woh_d = nc.dram_tensor("woh", [P, CC, H1], MDT, kind="ExternalInput")
    out_d = nc.dram_tensor("out", [C, N], F16, kind="ExternalOutput")

    wk_r = wk_d.rearrange("j p (k m) -> j p k m", m=P)
    out_r = out_d.rearrange("(k p) n -> p k n", p=P)

    with tile.TileContext(nc) as tc:
        with (
            tc.tile_pool(name="consts", bufs=1) as cpool,
            tc.tile_pool(name="acts", bufs=1) as apool,
            tc.tile_pool(name="s1", bufs=1) as s1pool,
            tc.tile_pool(name="pt", bufs=4) as ptpool,
            tc.tile_pool(name="rec", bufs=4) as rpool,
            tc.tile_pool(name="outsb", bufs=3) as opool,
            tc.tile_pool(name="pst", bufs=2, space="PSUM") as pst,
            tc.tile_pool(name="pmm", bufs=2, space="PSUM") as pmm,
        ):
            # ---- SBUF input tiles ----
            x_sb = s1pool.tile([P, CC, N], MDT)
            wql_sb = s1pool.tile([P, CC, H1], MDT)
            wqh_sb = s1pool.tile([P, CC, H1], MDT)
            ctxT_sb = s1pool.tile([P, KC, T], MDT)
            wk_sb = s1pool.tile([P, CC, KC, P], MDT)   # (p, jc, kc, m)
            wvl_sb = s1pool.tile([P, KC, H1], MDT)
            wvh_sb = s1pool.tile([P, KC, H1], MDT)
            wol_sb = s1pool.tile([P, CC, H1], MDT)
            woh_sb = s1pool.tile([P, CC, H1], MDT)
            mb_sb = cpool.tile([P, TC], F32)

            # ---- DMA issue order (per-queue order = arrival order;
            # pieces ordered by PE consumption: kT, qT-a, qT-b, fills, out)
            wk_pr = wk_d.rearrange("j p (k m) -> p j k m", m=P)
            with tc.high_priority():
                nc.sync.dma_start(wk_sb[:, 0:1, :, :], wk_pr[:, 0:1])
                nc.scalar.dma_start(wk_sb[:, 1:2, :, :], wk_pr[:, 1:2])
                nc.sync.dma_start(ctxT_sb[:, 0:4, :], ctxT_d[:, 0:4, :])
                nc.scalar.dma_start(ctxT_sb[:, 4:8, :], ctxT_d[:, 4:8, :])
                nc.sync.dma_start(wk_sb[:, 2:4, :, :], wk_pr[:, 2:4])
                nc.scalar.dma_start(wk_sb[:, 4:6, :, :], wk_pr[:, 4:6])
                nc.sync.dma_start(x_sb[:, 0:3, :], x_d[:, 0:3, :])
                nc.scalar.dma_start(wql_sb[:], wql_d[:])
                nc.scalar.dma_start(x_sb[:, 3:6, :], x_d[:, 3:6, :])
                nc.sync.dma_start(wvl_sb[:], wvl_d[:])
                nc.scalar.dma_start(wvh_sb[:], wvh_d[:])
                nc.sync.dma_start(wqh_sb[:], wqh_d[:])
                nc.scalar.dma_start(woh_sb[:], woh_d[:])
                nc.sync.dma_start(wol_sb[:], wol_d[:])
                nc.gpsimd.dma_start(mb_sb[:], mb_d[:])

            # ---- activation tiles ----
            qT = apool.tile([P, CC, N], MDT)
            kT = apool.tile([P, CC, T], MDT)
            vT = apool.tile([P, TC, NH, D], MDT)
            oT = apool.tile([P, CC, N], MDT)
            ones_sb = cpool.tile([P, D], MDT)
            nc.gpsimd.memset(ones_sb[:], 1.0)

            # ---- PE warm-up junk ----
            warm = cpool.tile([P, 256], MDT)
            nc.vector.memset(warm[:], 0.0)
            for w in range(22):
                wps = pmm.tile([P, N], F32, tag="mm", name=f"warm{w % 4}")
                nc.tensor.matmul(wps[:, 0:256], warm[:, 0:P], warm[:],
                                 start=True, stop=True)

            # ---- stage helpers (self-contained psum units) ----
            def ptile(pool, name):
                return pool.tile([P, N], F32,
                                 tag="mm" if pool is pmm else "st", name=name)

            def kt_chunk(jc, pool=None):
                ps = ptile(pool or pmm, f"k{jc}")
                for kc in range(KC):
                    nc.tensor.matmul(
                        ps[:, 0:T],
                        wk_sb[:, jc, kc, :],
                        ctxT_sb[:, kc, :],
                        start=(kc == 0),
                        stop=(kc == KC - 1),
                    )
                nc.vector.tensor_copy(kT[:, jc, :], ps[:, 0:T])

            def q_mms(ps, jc, nh, kcs):
                wq_sb = wql_sb if jc < 3 else wqh_sb
                jcl = jc % 3
                for kc in kcs:
                    nc.tensor.matmul(
                        ps[:, 0:512],
                        wq_sb[:, kc, jcl * P:(jcl + 1) * P],
                        x_sb[:, kc, nh * 512:(nh + 1) * 512],
                        start=(kc == 0),
                        stop=(kc == CC - 1),
                    )

            def q_half(jc, nh, pool=None):
                # qT[:, jc, nh*512:(nh+1)*512], 6 kc-accumulating mms
                ps = ptile(pool or pmm, f"q{jc}_{nh}")
                q_mms(ps, jc, nh, range(CC))
                nc.vector.tensor_copy(qT[:, jc, nh * 512:(nh + 1) * 512],
                                      ps[:, 0:512])

            def v_unit(jh, tcc, pool=None):
                # heads 6jh..6jh+5, one t-chunk
                ps = ptile(pool or pmm, f"v{jh}_{tcc}")
                for kc in range(KC):
                    nc.tensor.matmul(
                        ps[:, 0:H1],
                        ctxT_sb[:, kc, tcc * P:(tcc + 1) * P],
                        (wvl_sb if jh == 0 else wvh_sb)[:, kc, :],
                        start=(kc == 0),
                        stop=(kc == KC - 1),
                    )
                nc.vector.tensor_copy(
                    vT[:, tcc, 6 * jh:6 * jh + 6, :],
                    ps[:, 0:H1].rearrange("p (h d) -> p h d", d=D),
                )

            # fill units consumed inside the attention pair loop
            fills = [
                lambda: v_unit(0, 0),
                lambda: v_unit(0, 1),
                lambda: q_half(3, 0),
                lambda: q_half(3, 1),
                lambda: v_unit(1, 0),
                lambda: v_unit(1, 1),
                lambda: q_half(4, 0),
                lambda: q_half(4, 1),
                lambda: q_half(5, 0),
                lambda: q_half(5, 1),
            ]
            FILL_PLAN = [2, 2, 2, 2, 2, 0]   # units consumed per pair
            fi = [0]

            def fill(k=1):
                for _ in range(k):
                    if fi[0] < len(fills):
                        fills[fi[0]]()
                        fi[0] += 1

            def st_stage(p, tcc, pts):
                # per-head score tiles; the two heads' K=64 mms issue
                # adjacently at PE row offsets 0/64 so they co-execute
                sts = [pst.tile([P, N], F32, tag="st", name=f"st{hl}")
                       for hl in range(2)]
                for nh in range(2):
                    for hl in range(2):
                        r0 = hl * D
                        nc.tensor.matmul(
                            sts[hl][:, nh * 512:(nh + 1) * 512],
                            kT[r0:r0 + D, p, tcc * P:(tcc + 1) * P],
                            qT[r0:r0 + D, p, nh * 512:(nh + 1) * 512],
                            start=True,
                            stop=True,
                        )
                for hl in range(2):
                    nc.scalar.activation(
                        pts[hl][:, tcc, :],
                        sts[hl][:],
                        mybir.ActivationFunctionType.Exp,
                        bias=mb_sb[:, tcc:tcc + 1],
                    )

            def denotp_stage(p, pts):
                # denominators: den[hl*64:...] = ones.T @ PT_hl — col-tiled
                # pair (positions (0,0)/(0,64)) sharing banks row-disjoint
                den = pmm.tile([P, N], F32, tag="mm", name="den")
                for tcc in range(TC):
                    for nh in range(2):
                        for hl in range(2):
                            nc.tensor.matmul(
                                den[hl * D:(hl + 1) * D, nh * 512:(nh + 1) * 512],
                                ones_sb[:],
                                pts[hl][:, tcc, nh * 512:(nh + 1) * 512],
                                start=(tcc == 0),
                                stop=(tcc == TC - 1),
                            )
                # raw attention outputs, col-tiled the same way
                otp = pmm.tile([P, N], F32, tag="mm", name="otp")
                for tcc in range(TC):
                    for nh in range(2):
                        for hl in range(2):
                            nc.tensor.matmul(
                                otp[hl * D:(hl + 1) * D, nh * 512:(nh + 1) * 512],
                                vT[:, tcc, 2 * p + hl, :],
                                pts[hl][:, tcc, nh * 512:(nh + 1) * 512],
                                start=(tcc == 0),
                                stop=(tcc == TC - 1),
                            )
                # one full-lane reciprocal + one full-lane multiply per pair
                rec = rpool.tile([P, N], F32, tag="rec")
                nc.vector.reciprocal_approx_fast(rec[:], den[:])
                nc.vector.tensor_mul(oT[:, p, :], otp[:], rec[:])

            # ---- pre-attention: kT, qT jc0-2 — units alternate between
            # the two psum pools (pst is idle until attention) so the
            # copy+sem latency of unit i never stalls unit i+2
            POOLS = [pmm, pst]
            for jc in range(CC):
                kt_chunk(jc, pool=POOLS[jc % 2])
            # q jc0/jc1: four tiles, kc0-2 mms first (x kc0-2 lands
            # earlier), then kc3-5 + copies — keeps PE fed while x streams
            qtiles = {}
            for ui, (jc, nh) in enumerate([(0, 0), (0, 1), (1, 0), (1, 1)]):
                ps = ptile(POOLS[ui % 2], f"q{jc}_{nh}")
                qtiles[(jc, nh)] = ps
                q_mms(ps, jc, nh, range(3))
            for ui, (jc, nh) in enumerate([(0, 0), (0, 1), (1, 0), (1, 1)]):
                ps = qtiles[(jc, nh)]
                q_mms(ps, jc, nh, range(3, CC))
                nc.vector.tensor_copy(qT[:, jc, nh * 512:(nh + 1) * 512],
                                      ps[:, 0:512])
            q_half(2, 0, pool=pmm)
            q_half(2, 1, pool=pst)

            # ACT exp-table preload: walrus inserts the table load before the
            # first ACTIVATE; keep that off the DMA-issue window
            dume = cpool.tile([P, 8], MDT)
            nc.scalar.activation(dume[:], warm[:, 0:8],
                                 mybir.ActivationFunctionType.Exp)

            # ---- attention: 6 head-pairs, software-pipelined so pair p-1's
            # den/otp matmuls run under pair p's exp windows ----
            pending = None
            for p in range(CC):
                pts = [ptpool.tile([P, TC, N], MDT, tag="pt",
                                   name=f"pt{(2 * p + hl) % 4}")
                       for hl in range(2)]
                st_stage(p, 0, pts)
                if pending is not None:
                    denotp_stage(*pending)
                fill(1)
                st_stage(p, 1, pts)
                fill(FILL_PLAN[p] - 1)
                pending = (p, pts)
            denotp_stage(*pending)

            # ---- out projection: outT = Wo.T @ oT ----
            # last oc as two separate 1-bank half-tiles so the final
            # copy+DMA tail overlaps the last matmuls
            oqs = [nc.sync, nc.scalar]
            for oc in range(CC - 1):
                ps = pmm.tile([P, N], F32, tag="mm", name=f"o{oc % 2}")
                for jc in range(CC):
                    for nh in range(2):
                        nc.tensor.matmul(
                            ps[:, nh * 512:(nh + 1) * 512],
                            (wol_sb if oc < 3 else woh_sb)[
                                :, jc, (oc % 3) * P:(oc % 3 + 1) * P],
                            oT[:, jc, nh * 512:(nh + 1) * 512],
                            start=(jc == 0),
                            stop=(jc == CC - 1),
                        )
                ob = opool.tile([P, N], F16, tag="ob")
                nc.scalar.copy(ob[:, 0:512], ps[:, 0:512])
                nc.vector.tensor_copy(ob[:, 512:1024], ps[:, 512:1024])
                oqs[oc % 2].dma_start(out_r[:, oc, 0:512], ob[:, 0:512])
                oqs[(oc + 1) % 2].dma_start(
                    out_r[:, oc, 512:1024], ob[:, 512:1024])
            oc = CC - 1
            ob = opool.tile([P, N], F16, tag="ob")
            for nh in range(2):
                ph = pmm.tile([P, 512], F32, tag="mm", name=f"oh{nh}")
                for jc in range(CC):
                    nc.tensor.matmul(
                        ph[:],
                        woh_sb[:, jc, (oc % 3) * P:(oc % 3 + 1) * P],
                        oT[:, jc, nh * 512:(nh + 1) * 512],
                        start=(jc == 0),
                        stop=(jc == CC - 1),
                    )
                sl = slice(nh * 512, (nh + 1) * 512)
                if nh == 0:
                    nc.scalar.copy(ob[:, sl], ph[:])
                else:
                    nc.vector.tensor_copy(ob[:, sl], ph[:])
                oqs[nh].dma_start(out_r[:, oc, sl], ob[:, sl])

    nc.compile()
    _NC_CACHE["nc"] = nc
    return nc


def kernel(x, context, context_mask, Wq, bq, Wk, bk, Wv, bv, Wo, bo,
           _trace=False):
    np_dt = np.float16
    x = np.asarray(x, dtype=np.float32)
    context = np.asarray(context, dtype=np.float32)
    context_mask = np.asarray(context_mask)
    scale = float(D) ** -0.5

    def pk(w, rows):     # [rows*128, 768] -> [128, rows, 768] partition-major
        return np.asarray(w).reshape(rows, P, C).transpose(1, 0, 2)

    wq_p = pk(np.asarray(Wq) * scale, CC)
    wql = np.ascontiguousarray(wq_p[:, :, 0:384]).astype(np_dt)
    wqh = np.ascontiguousarray(wq_p[:, :, 384:768]).astype(np_dt)
    # wk permuted to [jc, p, kc*128]: wk_p[j, p, k*128+m] = Wk[k*128+p, j*128+m]
    wk = np.ascontiguousarray(
        np.asarray(Wk).reshape(KC, P, CC, P).transpose(2, 1, 0, 3)
        .reshape(CC, P, CTX)).astype(np_dt)
    wv_p = pk(Wv, KC)
    wvl = np.ascontiguousarray(wv_p[:, :, 0:384]).astype(np_dt)
    wvh = np.ascontiguousarray(wv_p[:, :, 384:768]).astype(np_dt)
    wo_p = pk(Wo, CC)
    wol = np.ascontiguousarray(wo_p[:, :, 0:384]).astype(np_dt)
    woh = np.ascontiguousarray(wo_p[:, :, 384:768]).astype(np_dt)

    in_maps = []
    for b in range(B):
        mb = np.where(context_mask[b] != 0, 0.0, MASK_NEG).astype(np.float32)
        im = {
            "x": np.ascontiguousarray(
                x[b].reshape(CC, P, N).transpose(1, 0, 2)).astype(np_dt),
            "ctxT": np.ascontiguousarray(
                context[b].T.reshape(KC, P, T).transpose(1, 0, 2)
            ).astype(np_dt),
            "mb": np.ascontiguousarray(mb.reshape(TC, P).T),
            "wql": wql, "wqh": wqh, "wk": wk,
            "wvl": wvl, "wvh": wvh, "wol": wol, "woh": woh,
        }
        in_maps.append(im)

    nc = _build_nc()
    try:
        res = run_bass_kernel_spmd(nc, in_maps, list(range(B)), trace=_trace)
    except Exception:
        # transient NRT_EXEC_UNIT_UNRECOVERABLE etc. — one retry
        res = run_bass_kernel_spmd(nc, in_maps, list(range(B)), trace=_trace)
    out = np.stack([
        res.results[b]["out"].astype(np.float32).reshape(C, 32, 32)
        for b in range(B)
    ])
    if _trace:
        kernel.last_exec_time_ns = res.exec_time_ns
        kernel.last_results = res
    return out


# revision 25
# speedup vs baseline: 1.1471x; 1.0307x over previous
"""CrossAttention2D TRN2 Bass kernel — data-parallel over batch on 8 NeuronCores.

Per core (one batch element), computed fully transposed ("feature-major"):
  qT[j,n]  = Wq'[c,j].T @ x[c,n]            (Wq' = Wq*scale)
  kT[j,t]  = Wk[c,j].T @ ctxT[c,t]
  v[t,j]   = ctxT[c,t].T @ Wv[c,j]
  ST[t,n]  = kT_h[d,t].T @ qT_h[d,n]        (per head, K=64)
  PT[t,n]  = exp(ST + maskbias[t])          (mask folds into ACT per-partition bias)
  OT       = [1(x64) | v_h][t,128].T @ PT[t,n]
             rows 0:64 = softmax denominator replicated, rows 64:128 = raw O
  oT[d,n]  = OT[64:128] * recip_fast(OT[0:64])   (one DVE op each)
  outT[co,n] = Wo[j,co].T @ oT[j,n]

v2 schedule (vs baseline):
  - Head pairs: the two K=64 ST matmuls of heads (2p, 2p+1) are issued
    back-to-back at PE row offsets 0/64 so they genuinely co-execute.
  - Fused exp: one ACTIVATE over [128, 2048] psum (both heads of a pair,
    one tcc) instead of two [128,1024] — cuts ACT fixed cost.
  - qT chunks jc3-5, v half 1, and v half 0 are interleaved INTO the
    attention pair loop as "fill units" so PE chews projection work while
    ACT exps run (attention phase is otherwise ACT-bound).
  - DMA: ctx+wk(jc01) first so kT starts ~10us; x/wq-lo interleaved per-kc
    so qT streams at arrival pace; wv/wq-hi/wo later (JIT deadlines).
  - Copies routed off ACT during attention (DVE), on ACT elsewhere.
  - PSUM: "st" pool 1x[128,2048] (4 banks) + "mm" pool 2x[128,1024] (4 banks).
Output fp16, upcast on host.
"""

import sys

for _p in ("/opt/trn_rl_repo", "/opt/pypackages"):
    if _p not in sys.path:
        sys.path.append(_p)

import numpy as np

import concourse.bass as bass
import concourse.tile as tile
from concourse import bacc, mybir
from concourse.bass_utils import run_bass_kernel_spmd

F32 = mybir.dt.float32
F16 = mybir.dt.float16

B = 8
C = 768            # DIM (q/out features)
CTX = 1024         # CTX_DIM
T = 256            # context tokens
N = 1024           # image tokens (32*32)
NH = 12            # heads
D = 64             # head dim
P = 128
CC = C // P        # 6
TC = T // P        # 2
KC = CTX // P      # 8
MASK_NEG = -30.0

_NC_CACHE = {}


def _build_nc():
    if "nc" in _NC_CACHE:
        return _NC_CACHE["nc"]

    MDT = F16

    nc = bacc.Bacc("TRN2", target_bir_lowering=False, debug=False)

    # ALL inputs host-packed partition-major so every DMA piece is a
    # contiguous >=2KB run per partition on both DRAM and SBUF sides
    x_d = nc.dram_tensor("x", [P, CC, N], MDT, kind="ExternalInput")
    ctxT_d = nc.dram_tensor("ctxT", [P, KC, T], MDT, kind="ExternalInput")
    mb_d = nc.dram_tensor("mb", [P, TC], F32, kind="ExternalInput")
    H1 = 384
    wql_d = nc.dram_tensor("wql", [P, CC, H1], MDT, kind="ExternalInput")
    wqh_d = nc.dram_tensor("wqh", [P, CC, H1], MDT, kind="ExternalInput")
    wk_d = nc.dram_tensor("wk", [CC, P, CTX], MDT, kind="ExternalInput")
    wvl_d = nc.dram_tensor("wvl", [P, KC, H1], MDT, kind="ExternalInput")
    wvh_d = nc.dram_tensor("wvh", [P, KC, H1], MDT, kind="ExternalInput")
    wol_d = nc.dram_tensor("wol", [P, CC, H1], MDT, kind="ExternalInput")
    woh_d = nc.dram_tensor("woh", [P, CC, H1], MDT, kind="ExternalInput")
    out_d = nc.dram_tensor("out", [C, N], F16, kind="ExternalOutput")

    wk_r = wk_d.rearrange("j p (k m) -> j p k m", m=P)
    out_r = out_d.rearrange("(k p) n -> p k n", p=P)

    with tile.TileContext(nc) as tc:
        with (
            tc.tile_pool(name="consts", bufs=1) as cpool,
            tc.tile_pool(name="acts", bufs=1) as apool,
            tc.tile_pool(name="s1", bufs=1) as s1pool,
            tc.tile_pool(name="pt", bufs=4) as ptpool,
            tc.tile_pool(name="rec", bufs=4) as rpool,
            tc.tile_pool(name="outsb", bufs=3) as opool,
            tc.tile_pool(name="pst", bufs=2, space="PSUM") as pst,
            tc.tile_pool(name="pmm", bufs=2, space="PSUM") as pmm,
        ):
            # ---- SBUF input tiles ----
            x_sb = s1pool.tile([P, CC, N], MDT)
            wql_sb = s1pool.tile([P, CC, H1], MDT)
            wqh_sb = s1pool.tile([P, CC, H1], MDT)
            ctxT_sb = s1pool.tile([P, KC, T], MDT)
            wk_sb = s1pool.tile([P, CC, KC, P], MDT)   # (p, jc, kc, m)
            wvl_sb = s1pool.tile([P, KC, H1], MDT)
            wvh_sb = s1pool.tile([P, KC, H1], MDT)
            wol_sb = s1pool.tile([P, CC, H1], MDT)
            woh_sb = s1pool.tile([P, CC, H1], MDT)
            mb_sb = cpool.tile([P, TC], F32)

            # ---- DMA issue order (per-queue order = arrival order;
            # pieces ordered by PE consumption: kT, qT-a, qT-b, fills, out)
            wk_pr = wk_d.rearrange("j p (k m) -> p j k m", m=P)
            with tc.high_priority():
                nc.sync.dma_start(wk_sb[:, 0:1, :, :], wk_pr[:, 0:1])
                nc.scalar.dma_start(wk_sb[:, 1:2, :, :], wk_pr[:, 1:2])
                nc.sync.dma_start(ctxT_sb[:, 0:4, :], ctxT_d[:, 0:4, :])
                nc.scalar.dma_start(ctxT_sb[:, 4:8, :], ctxT_d[:, 4:8, :])
                nc.sync.dma_start(wk_sb[:, 2:4, :, :], wk_pr[:, 2:4])
                nc.scalar.dma_start(wk_sb[:, 4:6, :, :], wk_pr[:, 4:6])
                nc.sync.dma_start(x_sb[:, 0:3, :], x_d[:, 0:3, :])
                nc.scalar.dma_start(wql_sb[:], wql_d[:])
                nc.scalar.dma_start(x_sb[:, 3:6, :], x_d[:, 3:6, :])
                nc.sync.dma_start(wvl_sb[:], wvl_d[:])
                nc.scalar.dma_start(wvh_sb[:], wvh_d[:])
                nc.sync.dma_start(wqh_sb[:], wqh_d[:])
                nc.scalar.dma_start(woh_sb[:], woh_d[:])
                nc.sync.dma_start(wol_sb[:], wol_d[:])
                nc.gpsimd.dma_start(mb_sb[:], mb_d[:])

            # ---- activation tiles ----
            qT = apool.tile([P, CC, N], MDT)
            kT = apool.tile([P, CC, T], MDT)
            vT = apool.tile([P, TC, NH, D], MDT)
            oT = apool.tile([P, CC, N], MDT)
            ones_sb = cpool.tile([P, D], MDT)
            nc.gpsimd.memset(ones_sb[:], 1.0)

            # ---- PE warm-up junk ----
            warm = cpool.tile([P, 256], MDT)
            nc.vector.memset(warm[:], 0.0)
            for w in range(22):
                wps = pmm.tile([P, N], F32, tag="mm", name=f"warm{w % 4}")
                nc.tensor.matmul(wps[:, 0:256], warm[:, 0:P], warm[:],
                                 start=True, stop=True)

            # ---- stage helpers (self-contained psum units) ----
            def ptile(pool, name):
                return pool.tile([P, N], F32,
                                 tag="mm" if pool is pmm else "st", name=name)

            def kt_chunk(jc, pool=None):
                ps = ptile(pool or pmm, f"k{jc}")
                for kc in range(KC):
                    nc.tensor.matmul(
                        ps[:, 0:T],
                        wk_sb[:, jc, kc, :],
                        ctxT_sb[:, kc, :],
                        start=(kc == 0),
                        stop=(kc == KC - 1),
                    )
                nc.vector.tensor_copy(kT[:, jc, :], ps[:, 0:T])

            def q_mms(ps, jc, nh, kcs):
                wq_sb = wql_sb if jc < 3 else wqh_sb
                jcl = jc % 3
                for kc in kcs:
                    nc.tensor.matmul(
                        ps[:, 0:512],
                        wq_sb[:, kc, jcl * P:(jcl + 1) * P],
                        x_sb[:, kc, nh * 512:(nh + 1) * 512],
                        start=(kc == 0),
                        stop=(kc == CC - 1),
                    )

            def q_half(jc, nh, pool=None):
                # qT[:, jc, nh*512:(nh+1)*512], 6 kc-accumulating mms
                ps = ptile(pool or pmm, f"q{jc}_{nh}")
                q_mms(ps, jc, nh, range(CC))
                nc.vector.tensor_copy(qT[:, jc, nh * 512:(nh + 1) * 512],
                                      ps[:, 0:512])

            def v_unit(jh, tcc, pool=None):
                # heads 6jh..6jh+5, one t-chunk
                ps = ptile(pool or pmm, f"v{jh}_{tcc}")
                for kc in range(KC):
                    nc.tensor.matmul(
                        ps[:, 0:H1],
                        ctxT_sb[:, kc, tcc * P:(tcc + 1) * P],
                        (wvl_sb if jh == 0 else wvh_sb)[:, kc, :],
                        start=(kc == 0),
                        stop=(kc == KC - 1),
                    )
                nc.vector.tensor_copy(
                    vT[:, tcc, 6 * jh:6 * jh + 6, :],
                    ps[:, 0:H1].rearrange("p (h d) -> p h d", d=D),
                )

            # fill units consumed inside the attention pair loop
            fills = [
                lambda: q_half(3, 0),
                lambda: q_half(3, 1),
                lambda: q_half(4, 0),
                lambda: q_half(4, 1),
                lambda: q_half(5, 0),
                lambda: q_half(5, 1),
            ]
            FILL_PLAN = [1, 1, 1, 1, 2, 0]   # units consumed per pair
            fi = [0]

            def fill(k=1):
                for _ in range(k):
                    if fi[0] < len(fills):
                        fills[fi[0]]()
                        fi[0] += 1

            def st_stage(p, tcc, pts):
                # per-head score tiles; the two heads' K=64 mms issue
                # adjacently at PE row offsets 0/64 so they co-execute
                sts = [pst.tile([P, N], F32, tag="st", name=f"st{hl}")
                       for hl in range(2)]
                for nh in range(2):
                    for hl in range(2):
                        r0 = hl * D
                        nc.tensor.matmul(
                            sts[hl][:, nh * 512:(nh + 1) * 512],
                            kT[r0:r0 + D, p, tcc * P:(tcc + 1) * P],
                            qT[r0:r0 + D, p, nh * 512:(nh + 1) * 512],
                            start=True,
                            stop=True,
                        )
                for hl in range(2):
                    nc.scalar.activation(
                        pts[hl][:, tcc, :],
                        sts[hl][:],
                        mybir.ActivationFunctionType.Exp,
                        bias=mb_sb[:, tcc:tcc + 1],
                    )

            def denotp_stage(p, pts):
                # denominators: den[hl*64:...] = ones.T @ PT_hl — col-tiled
                # pair (positions (0,0)/(0,64)) sharing banks row-disjoint
                den = pmm.tile([P, N], F32, tag="mm", name="den")
                for tcc in range(TC):
                    for nh in range(2):
                        for hl in range(2):
                            nc.tensor.matmul(
                                den[hl * D:(hl + 1) * D, nh * 512:(nh + 1) * 512],
                                ones_sb[:],
                                pts[hl][:, tcc, nh * 512:(nh + 1) * 512],
                                start=(tcc == 0),
                                stop=(tcc == TC - 1),
                            )
                # raw attention outputs, col-tiled the same way
                otp = pmm.tile([P, N], F32, tag="mm", name="otp")
                for tcc in range(TC):
                    for nh in range(2):
                        for hl in range(2):
                            nc.tensor.matmul(
                                otp[hl * D:(hl + 1) * D, nh * 512:(nh + 1) * 512],
                                vT[:, tcc, 2 * p + hl, :],
                                pts[hl][:, tcc, nh * 512:(nh + 1) * 512],
                                start=(tcc == 0),
                                stop=(tcc == TC - 1),
                            )
                # one full-lane reciprocal + one full-lane multiply per pair
                rec = rpool.tile([P, N], F32, tag="rec")
                nc.vector.reciprocal_approx_fast(rec[:], den[:])
                nc.vector.tensor_mul(oT[:, p, :], otp[:], rec[:])

            # ---- pre-attention: kT, qT jc0-2 — units alternate between
            # the two psum pools (pst is idle until attention) so the
            # copy+sem latency of unit i never stalls unit i+2
            POOLS = [pmm, pst]
            for jc in range(CC):
                kt_chunk(jc, pool=POOLS[jc % 2])
            # q jc0/jc1: four tiles, kc0-2 mms first (x kc0-2 lands
            # earlier), then kc3-5 + copies — keeps PE fed while x streams
            qtiles = {}
            for ui, (jc, nh) in enumerate([(0, 0), (0, 1), (1, 0), (1, 1)]):
                ps = ptile(POOLS[ui % 2], f"q{jc}_{nh}")
                qtiles[(jc, nh)] = ps
                q_mms(ps, jc, nh, range(3))
            for ui, (jc, nh) in enumerate([(0, 0), (0, 1), (1, 0), (1, 1)]):
                ps = qtiles[(jc, nh)]
                q_mms(ps, jc, nh, range(3, CC))
                nc.vector.tensor_copy(qT[:, jc, nh * 512:(nh + 1) * 512],
                                      ps[:, 0:512])
            q_half(2, 0, pool=pmm)
            q_half(2, 1, pool=pst)
            v_unit(0, 0, pool=pmm)
            v_unit(0, 1, pool=pst)
            v_unit(1, 0, pool=pmm)
            v_unit(1, 1, pool=pst)

            # ACT exp-table preload: walrus inserts the table load before the
            # first ACTIVATE; keep that off the DMA-issue window
            dume = cpool.tile([P, 8], MDT)
            nc.scalar.activation(dume[:], warm[:, 0:8],
                                 mybir.ActivationFunctionType.Exp)

            # ---- attention: 6 head-pairs, software-pipelined so pair p-1's
            # den/otp matmuls run under pair p's exp windows ----
            pending = None
            for p in range(CC):
                pts = [ptpool.tile([P, TC, N], MDT, tag="pt",
                                   name=f"pt{(2 * p + hl) % 4}")
                       for hl in range(2)]
                st_stage(p, 0, pts)
                fill(FILL_PLAN[p])
                st_stage(p, 1, pts)
                if pending is not None:
                    denotp_stage(*pending)
                pending = (p, pts)
            denotp_stage(*pending)

            # ---- out projection: outT = Wo.T @ oT ----
            # last oc as two separate 1-bank half-tiles so the final
            # copy+DMA tail overlaps the last matmuls
            oqs = [nc.sync, nc.scalar]
            for oc in range(CC - 1):
                ps = pmm.tile([P, N], F32, tag="mm", name=f"o{oc % 2}")
                for jc in range(CC):
                    for nh in range(2):
                        nc.tensor.matmul(
                            ps[:, nh * 512:(nh + 1) * 512],
                            (wol_sb if oc < 3 else woh_sb)[
                                :, jc, (oc % 3) * P:(oc % 3 + 1) * P],
                            oT[:, jc, nh * 512:(nh + 1) * 512],
                            start=(jc == 0),
                            stop=(jc == CC - 1),
                        )
                ob = opool.tile([P, N], F16, tag="ob")
                nc.scalar.copy(ob[:, 0:512], ps[:, 0:512])
                nc.vector.tensor_copy(ob[:, 512:1024], ps[:, 512:1024])
                oqs[oc % 2].dma_start(out_r[:, oc, 0:512], ob[:, 0:512])
                oqs[(oc + 1) % 2].dma_start(
                    out_r[:, oc, 512:1024], ob[:, 512:1024])
            oc = CC - 1
            ob = opool.tile([P, N], F16, tag="ob")
            for nh in range(2):
                ph = pmm.tile([P, 512], F32, tag="mm", name=f"oh{nh}")
                for jc in range(CC):
                    nc.tensor.matmul(
                        ph[:],
                        woh_sb[:, jc, (oc % 3) * P:(oc % 3 + 1) * P],
                        oT[:, jc, nh * 512:(nh + 1) * 512],
                        start=(jc == 0),
                        stop=(jc == CC - 1),
                    )
                sl = slice(nh * 512, (nh + 1) * 512)
                if nh == 0:
                    nc.scalar.copy(ob[:, sl], ph[:])
                else:
                    nc.vector.tensor_copy(ob[:, sl], ph[:])
                oqs[nh].dma_start(out_r[:, oc, sl], ob[:, sl])

    nc.compile()
    _NC_CACHE["nc"] = nc
    return nc


def kernel(x, context, context_mask, Wq, bq, Wk, bk, Wv, bv, Wo, bo,
           _trace=False):
    np_dt = np.float16
    x = np.asarray(x, dtype=np.float32)
    context = np.asarray(context, dtype=np.float32)
    context_mask = np.asarray(context_mask)
    scale = float(D) ** -0.5

    def pk(w, rows):     # [rows*128, 768] -> [128, rows, 768] partition-major
        return np.asarray(w).reshape(rows, P, C).transpose(1, 0, 2)

    wq_p = pk(np.asarray(Wq) * scale, CC)
    wql = np.ascontiguousarray(wq_p[:, :, 0:384]).astype(np_dt)
    wqh = np.ascontiguousarray(wq_p[:, :, 384:768]).astype(np_dt)
    # wk permuted to [jc, p, kc*128]: wk_p[j, p, k*128+m] = Wk[k*128+p, j*128+m]
    wk = np.ascontiguousarray(
        np.asarray(Wk).reshape(KC, P, CC, P).transpose(2, 1, 0, 3)
        .reshape(CC, P, CTX)).astype(np_dt)
    wv_p = pk(Wv, KC)
    wvl = np.ascontiguousarray(wv_p[:, :, 0:384]).astype(np_dt)
    wvh = np.ascontiguousarray(wv_p[:, :, 384:768]).astype(np_dt)
    wo_p = pk(Wo, CC)
    wol = np.ascontiguousarray(wo_p[:, :, 0:384]).astype(np_dt)
    woh = np.ascontiguousarray(wo_p[:, :, 384:768]).astype(np_dt)

    in_maps = []
    for b in range(B):
        mb = np.where(context_mask[b] != 0, 0.0, MASK_NEG).astype(np.float32)
        im = {
            "x": np.ascontiguousarray(
                x[b].reshape(CC, P, N).transpose(1, 0, 2)).astype(np_dt),
            "ctxT": np.ascontiguousarray(
                context[b].T.reshape(KC, P, T).transpose(1, 0, 2)
            ).astype(np_dt),
            "mb": np.ascontiguousarray(mb.reshape(TC, P).T),
            "wql": wql, "wqh": wqh, "wk": wk,
            "wvl": wvl, "wvh": wvh, "wol": wol, "woh": woh,
        }
        in_maps.append(im)

    nc = _build_nc()
    try:
        res = run_bass_kernel_spmd(nc, in_maps, list(range(B)), trace=_trace)
    except Exception:
        # transient NRT_EXEC_UNIT_UNRECOVERABLE etc. — one retry
        res = run_bass_kernel_spmd(nc, in_maps, list(range(B)), trace=_trace)
    out = np.stack([
        res.results[b]["out"].astype(np.float32).reshape(C, 32, 32)
        for b in range(B)
    ])
    if _trace:
        kernel.last_exec_time_ns = res.exec_time_ns
        kernel.last_results = res
    return out
